# revision 63
# baseline (speedup 1.0000x reference)
"""CktGNN encoder kernel for Trainium2 (Bass/Tile), 8-core data parallel.

Per core (local batch BL=512 = 4 b-tiles of 128):
  - "L2" tensors: [128 b-partitions, 4*HS free] fp16 (r/z/n/h/Hin/G).
  - "L1" tensors: [hs-partitions, 512 b free] fp16 (transposed h/Hin used as
    matmul stationary operands; produced by PE transpose each step).
  - All matmuls fp16 (1 cyc/row on PE), fp32 PSUM accumulation.
  - Biases folded into matmuls via ones-rows in the stationary data stack.
  - Aggregation Hin_w = sum_u adj[:,w,u] * g_u split across four mechanisms:
      * PE diagonal-matmul accumulation (host-prebuilt diag(adj) streamed
        from HBM in ONE batched DMA per step),
      * DVE scalar_tensor_tensor chains into a partial tile,
      * GPSIMD (Pool) stt chains into a second partial tile,
      * partials + backbone edge (u=w-1, always 1) folded into the PSUM
        accumulator with identity matmuls; the final PSUM->SBUF fold runs on
        a configurable engine (default Pool).
"""
import sys
sys.path.insert(0, "/opt/trn_rl_repo")

import numpy as np
import concourse.bass as bass
import concourse.tile as tile
from concourse import mybir
from concourse.bass_utils import run_bass_kernel_spmd
from concourse.vector_clock import ScopedClock
from contextlib import ExitStack

F16 = mybir.dt.float16
F32 = mybir.dt.float32
AL = mybir.AluOpType
ACTF = mybir.ActivationFunctionType

B = 4096
NCORES = 8
BL = B // NCORES          # 512
NBT = BL // 128           # 4 b-tiles
MAXN = 32
NVT = 26
P9 = 9
XD = NVT + P9             # 35
HS = 301
EMB = 16
FEAT = 8
NZ = 56
FLAT = NBT * HS           # 1204
HALF = 2 * HS             # 602

# xh2 tile rows: [X(35); ones(35); zero pad 36:64; HinT2 64:109 (45);
# ones(109)].  HinT2 sits at 64 so the DVE copy/memset and the hn matmul
# K-block start at a legal base partition (0/32/64/96).
XROWS = 110
HIN2 = 64                 # row offset of HinT2 block in xh2

# hT2x tile rows: [hT2(45); pos 45:54 (9); ones(54)]
H2ROWS = 55
# head2 rows: [hT2(45); pad 45:64; Hd 64:72 (8); ones(72)]
HDROWS = 73

# hs tiling for transposes
HT = [(0, 128), (128, 256), (256, 301)]

# ---- masked-agg term split (tunable) ----
DVE_CAP = 4               # max pairs per step on DVE stt chains
POOL_CAP = 6              # max pairs per step on GPSIMD stt chains
DVE_CAP_LATE = 7          # caps for late steps (2-step-deep chain hoisting)
POOL_CAP_LATE = 11
LATE_W = 12
PURE_DVE_MAX = 2          # steps with <= this many masked terms skip PSUM agg
DVE_P1_FRAC = 0.7         # share of DVE terms emitted early (part 1)

CAP_WAITS = True          # split >1 sem waits onto NoOps (walrus quirk)
FOLD_ENG = "act"          # agg psum -> hinL2 fold: "pool" | "act" | "dve"

_patched = [False]


def _patch_tile_drain():
    """This walrus build only supports ONE sem wait on a Drain instruction.
    Split the kernel-tail drain's waits across several drains."""
    if _patched[0]:
        return
    _patched[0] = True

    def patched(self, tick_clock, wait_clock):
        drain_inst = self.nc.sync.drain()
        wait_clock.add_sem_waits(
            drain_inst.ins, ScopedClock({None: tick_clock.global_clock})
        )
        si = drain_inst.ins.sync_info
        waits = list(si.on_wait or [])
        if len(waits) > 1:
            si.on_wait = waits[:1]
            for w in waits[1:]:
                d2 = self.nc.sync.drain()
                si2 = d2.ins.sync_info
                if si2 is None:
                    d2.ins.sync_info = mybir.SyncInfo(on_wait=[w], on_update=[])
                else:
                    si2.on_wait = [w]
        self.nc.all_engine_barrier()
        popped = self.nc._tile_sem_poison_stack.pop()
        assert popped is self._sem_poison
        self.nc.clear_and_free_semaphores(list(self.sems.allocated().values()))
        self.nc.all_engine_barrier()

    tile.TileContext._drain_and_barrier = patched


def _assign(w):
    """Split masked agg terms u in [0, w-2] for target vertex w.
    Returns (pe_us, pool_us, dve1_us, dve2_us).
    The backbone edge u = w-1 is handled separately."""
    us = list(range(w - 1))
    n = len(us)
    if n <= PURE_DVE_MAX:
        return [], [], us, []
    dcap = DVE_CAP_LATE if w >= LATE_W else DVE_CAP
    pcap = POOL_CAP_LATE if w >= LATE_W else POOL_CAP
    ndve = min(dcap, max(n - pcap - 1, 0))
    npool = min(pcap, n - ndve)
    pe_us = us[:n - ndve - npool]
    pool_us = us[n - ndve - npool:n - ndve]
    dve_us = us[n - ndve:]
    nd1 = int(round(DVE_P1_FRAC * len(dve_us)))
    return pe_us, pool_us, dve_us[:nd1], dve_us[nd1:]


def _pe_terms():
    """Flat ordering of (w, u, c) for the host-built diag tensor, grouped by
    step.  Returns (terms, step_off, step_cnt)."""
    terms = []
    step_off = {}
    step_cnt = {}
    for w in range(2, MAXN):
        pe_us, _, _, _ = _assign(w)
        step_off[w] = len(terms)
        for u in pe_us:
            for c in range(NBT):
                terms.append((w, u, c))
        step_cnt[w] = len(terms) - step_off[w]
    return terms, step_off, step_cnt


def _prep_weights(inp):
    f16 = np.float16
    W = {}
    Wg, bg, Wm = inp["Wg"], inp["bg"], inp["Wm"]
    W_ih, W_hh = inp["W_ih"], inp["W_hh"]
    b_ih, b_hh = inp["b_ih"], inp["b_hh"]

    # gate/mapper moving operands [K, 301]; K-split matches hT0/hT1/hT2x
    # hT2x rows: [h 256:301 (45); pos (9); ones (1)] = 55
    W["w_g0"] = Wg[0:128].astype(f16)
    W["w_g1"] = Wg[128:256].astype(f16)
    W["w_g2"] = np.vstack([Wg[256:301], Wg[301:310], bg[None, :]]).astype(f16)
    W["w_m0"] = Wm[0:128].astype(f16)
    W["w_m1"] = Wm[128:256].astype(f16)
    W["w_m2"] = np.vstack(
        [Wm[256:301], Wm[301:310], np.zeros((1, HS), np.float32)]
    ).astype(f16)

    # r,z: rows 0:602.  K-block 0 matches xh2 rows (110):
    # [X(35); ones(35); pad 36:64; HinT2 64:109; ones(109, zero weight)]
    Wih_rz = W_ih[0:2 * HS]
    Whh_rz = W_hh[0:2 * HS]
    b_rz = (b_ih + b_hh)[0:2 * HS]
    z28 = np.zeros((28, 2 * HS), np.float32)
    W["w_rz0"] = np.vstack([
        Wih_rz.T, b_rz[None, :], z28,
        Whh_rz[:, 256:301].T, np.zeros((1, 2 * HS), np.float32),
    ]).astype(f16)                                   # [110, 602]
    W["w_rz1"] = Whh_rz[:, 0:128].T.astype(f16)
    W["w_rz2"] = Whh_rz[:, 128:256].T.astype(f16)

    # hn: rows 602:903.  K-blocks: HinT0, HinT1, xh2[64:110]=[HinT2; ones]
    Whh_n = W_hh[2 * HS:3 * HS]
    b_hhn = b_hh[2 * HS:3 * HS]
    W["w_hn0"] = Whh_n[:, 0:128].T.astype(f16)
    W["w_hn1"] = Whh_n[:, 128:256].T.astype(f16)
    W["w_hnx"] = np.vstack([
        np.zeros((64, HS), np.float32), Whh_n[:, 256:301].T, b_hhn[None, :]
    ]).astype(f16)                                   # [110, 301]; rows 64:110

    # df encoder
    W["w_d1"] = np.vstack([inp["Wd1"], inp["bd1"][None, :]]).astype(f16)  # [28,16]
    W["w_d2"] = np.vstack([inp["Wd2"], inp["bd2"][None, :]]).astype(f16)  # [17,8]

    # head over head2 rows: [h 256:301 (45); pad 19; Hd (8); ones (1)] = 73
    Whead = np.concatenate([inp["Wmu"], inp["Wlv"]], 1)   # [309, 112]
    bhead = np.concatenate([inp["bmu"], inp["blv"]])
    W["w_hd0"] = Whead[0:128].astype(f16)
    W["w_hd1"] = Whead[128:256].astype(f16)
    W["w_hd2"] = np.vstack([
        Whead[256:301], np.zeros((19, 112), np.float32),
        Whead[301:309], bhead[None, :]
    ]).astype(f16)                                   # [73, 112]
    return W


def _build(nc):
    din = {}

    def dram(name, shape, dt, out=False):
        t = nc.dram_tensor(name, list(shape), dt,
                           kind="ExternalOutput" if out else "ExternalInput")
        din[name] = t
        return t

    terms, step_off, step_cnt = _pe_terms()
    npe4 = max(len(terms), 1)
    xt = dram("xt", [MAXN, 65, BL], F16)            # X rows + ones(35) + pad
    post = dram("post", [MAXN, 10, BL], F16)        # pos one-hot + ones rows
    int_d = dram("int_", [MAXN, 128, FLAT], F16)    # host-gathered i_n + bias
    ones1 = dram("ones1", [1, BL], F16)
    adjt = dram("adjt", [NBT, 128, MAXN * MAXN], F32)
    diagall = dram("diagall", [128, npe4 * 128], F16)
    hdft = dram("hdft", [28, BL], F16)
    ident = dram("ident", [128, 128], F16)
    wnames = [
        ("w_g0", [128, HS]), ("w_g1", [128, HS]), ("w_g2", [H2ROWS, HS]),
        ("w_m0", [128, HS]), ("w_m1", [128, HS]), ("w_m2", [H2ROWS, HS]),
        ("w_rz0", [XROWS, 2 * HS]), ("w_rz1", [128, 2 * HS]),
        ("w_rz2", [128, 2 * HS]),
        ("w_hn0", [128, HS]), ("w_hn1", [128, HS]), ("w_hnx", [XROWS, HS]),
        ("w_d1", [28, EMB]), ("w_d2", [EMB + 1, FEAT]),
        ("w_hd0", [128, 112]), ("w_hd1", [128, 112]), ("w_hd2", [HDROWS, 112]),
    ]
    for n, s in wnames:
        dram(n, s, F16)
    out_d = dram("out", [NBT, 128, 112], F32, out=True)

    max_cnt = max(list(step_cnt.values()) + [1])

    with tile.TileContext(nc) as tc, ExitStack() as ctx:
        wp = ctx.enter_context(tc.tile_pool(name="w", bufs=1))
        xp = ctx.enter_context(tc.tile_pool(name="x", bufs=3))
        hp = ctx.enter_context(tc.tile_pool(name="h", bufs=2))
        sp = ctx.enter_context(tc.tile_pool(name="s", bufs=1))
        gp_ = ctx.enter_context(tc.tile_pool(name="g", bufs=1))
        dgp = ctx.enter_context(tc.tile_pool(name="dg", bufs=2))
        itp = ctx.enter_context(tc.tile_pool(name="it", bufs=2))
        pp = ctx.enter_context(tc.tile_pool(name="ps", bufs=2, space="PSUM"))
        ap_ = ctx.enter_context(tc.tile_pool(name="agps", bufs=2, space="PSUM"))

        wt = {}
        for n, s in wnames:
            t = wp.tile(list(s), F16, tag=n, name=f"wt_{n}")
            nc.sync.dma_start(t[:], din[n].ap()[:])
            wt[n] = t
        adj_t = []
        for c in range(NBT):
            t = wp.tile([128, MAXN * MAXN], F32, tag=f"adj{c}", name=f"adj{c}")
            nc.sync.dma_start(t[:], adjt.ap()[c])
            adj_t.append(t)
        id_t = wp.tile([128, 128], F16, tag="ident")
        nc.sync.dma_start(id_t[:], ident.ap()[:])
        hdft_t = wp.tile([28, BL], F16, tag="hdft")
        nc.sync.dma_start(hdft_t[:], hdft.ap()[:])

        g_tiles = [gp_.tile([128, FLAT], F16, tag=f"gv{u}", name=f"gv{u}")
                   for u in range(MAXN - 1)]

        # ---- df encoder ----
        ps_d = pp.tile([128, 1024], F32, tag="ps")
        nc.tensor.matmul(ps_d[0:EMB, 0:BL], wt["w_d1"][:], hdft_t[:],
                         start=True, stop=True)
        relu_t = wp.tile([EMB + 1, BL], F16, tag="relu")
        nc.vector.memset(relu_t[:], 1.0)
        nc.scalar.activation(relu_t[0:EMB, :], ps_d[0:EMB, 0:BL], ACTF.Relu)
        ps_d2 = pp.tile([128, 1024], F32, tag="ps")
        nc.tensor.matmul(ps_d2[0:FEAT, 0:BL], wt["w_d2"][:], relu_t[:],
                         start=True, stop=True)
        hdT_t = wp.tile([FEAT, BL], F16, tag="hdT")
        nc.scalar.activation(hdT_t[:], ps_d2[0:FEAT, 0:BL], ACTF.Copy)

        # ---- per-step helpers ----
        def new_xh2(v):
            t = xp.tile([XROWS, BL], F16, tag="xh2")
            nc.sync.dma_start(t[0:65, :], xt.ap()[v])
            nc.sync.dma_start(t[XROWS - 1:XROWS, :], ones1.ap()[:])
            return t

        def new_int(v):
            t = itp.tile([128, FLAT], F16, tag="int", name=f"int{v}")
            nc.sync.dma_start(t[:], int_d.ap()[v])
            return t

        def new_diag(w):
            """DMA this step's diag tiles in one shot (for agg of step w)."""
            cnt = step_cnt.get(w, 0)
            if cnt == 0:
                return None
            t = dgp.tile([128, max_cnt * 128], F16, tag="diag", name=f"dg{w}")
            off = step_off[w]
            nc.sync.dma_start(
                t[:, 0:cnt * 128],
                diagall.ap()[:, off * 128:(off + cnt) * 128])
            return t

        def pv2(t):
            """[128, 2, 301] view of a [128, 1024] psum tile's two slabs."""
            return t[:].rearrange("p (c w) -> p c w", c=2)[:, :, 0:HS]

        def sb2(t, j):
            """[128, 2, 301] view of half j of a dense [128, FLAT] tile."""
            return t[:, j * HALF:(j + 1) * HALF].rearrange(
                "p (c w) -> p c w", c=2)

        def fold_engine(eng):
            return {"pool": nc.gpsimd, "dve": nc.vector}.get(eng)

        xh2 = new_xh2(0)
        nc.vector.memset(xh2[HIN2:XROWS, :], 0.0)   # Hin(0)=0
        nc.sync.dma_start(xh2[XROWS - 1:XROWS, :], ones1.ap()[:])
        in_t = new_int(0)
        dg_cur = new_diag(2)   # diag for agg(2) (none: w=2 is pure-DVE)
        hinT0 = None
        hinT1 = None
        hinL2 = hp.tile([128, FLAT], F16, tag="hinl2")
        nc.vector.memset(hinL2[:], 0.0)

        hT0_last = hT1_last = head2 = None

        for v in range(MAXN):
            last = v == MAXN - 1
            w = v + 1

            # ---------- prefetches ----------
            if not last:
                xh2_next = new_xh2(w)
                in_t_next = new_int(w)
            dg_next = new_diag(w + 1) if not last else None

            if not last:
                pe_us, pool_us, dve1_us, dve2_us = _assign(w)
                psum_path = bool(pe_us or pool_us or dve2_us) or \
                    (len(dve1_us) > PURE_DVE_MAX)
            else:
                pe_us = pool_us = dve1_us = dve2_us = []
                psum_path = False

            # ---------- agg(w): PSUM alloc + early PE diag terms ----------
            if psum_path:
                agg_ps = [ap_.tile([128, 1024], F32, tag="agps",
                                   name=f"agg{j}_{v}") for j in range(2)]
                for ui, u in enumerate(pe_us):
                    for c in range(NBT):
                        dst = agg_ps[c // 2][:, (c % 2) * 512:(c % 2) * 512 + HS]
                        k = ui * NBT + c
                        nc.tensor.matmul(
                            dst, dg_cur[:, k * 128:(k + 1) * 128],
                            g_tiles[u][:, bass.ts(c, HS)],
                            start=(ui == 0), stop=False)
            else:
                agg_ps = None

            # ---------- agg(w): Pool chain (early) ----------
            pool_part = None
            if pool_us:
                pool_part = hp.tile([128, FLAT], F16, tag="poolpart",
                                    name=f"pp{v}")
                for c in range(NBT):
                    pslab = pool_part[:, bass.ts(c, HS)]
                    for i, u in enumerate(pool_us):
                        gsl = g_tiles[u][:, bass.ts(c, HS)]
                        sc = adj_t[c][:, w * MAXN + u: w * MAXN + u + 1]
                        if i == 0:
                            nc.gpsimd.tensor_scalar(pslab, gsl, sc, None,
                                                    AL.mult)
                        else:
                            nc.gpsimd.scalar_tensor_tensor(
                                pslab, gsl, sc, pslab, AL.mult, AL.add)

            # ---------- agg(w): DVE chain part 1 (early) ----------
            dve_part = None
            if psum_path and (dve1_us or dve2_us):
                dve_part = hp.tile([128, FLAT], F16, tag="dvepart",
                                   name=f"dp{v}")
                for c in range(NBT):
                    dslab = dve_part[:, bass.ts(c, HS)]
                    for i, u in enumerate(dve1_us):
                        gsl = g_tiles[u][:, bass.ts(c, HS)]
                        sc = adj_t[c][:, w * MAXN + u: w * MAXN + u + 1]
                        if i == 0:
                            nc.vector.tensor_scalar(dslab, gsl, sc, None,
                                                    AL.mult)
                        else:
                            nc.vector.scalar_tensor_tensor(
                                dslab, gsl, sc, dslab, AL.mult, AL.add)

            # ---------- GRU matmuls (r, hn interleaved; then z) -------
            rz_blocks = [(xh2[0:XROWS, :], "w_rz0", 0, XROWS)]
            hn_blocks = [(xh2[HIN2:XROWS, :], "w_hnx", HIN2, XROWS)]
            if v > 0:
                rz_blocks += [(hinT0[:], "w_rz1", 0, 128),
                              (hinT1[:], "w_rz2", 0, 128)]
                hn_blocks += [(hinT0[:], "w_hn0", 0, 128),
                              (hinT1[:], "w_hn1", 0, 128)]

            ps_r = [None, None]
            ps_hn = [None, None]
            ps_z = [None, None]

            def gate_mms(ps_pair, j, blocks, col0, col1, nm, pool=None):
                t = (pool or pp).tile([128, 1024], F32,
                                      tag="ps" if pool is None else "agps",
                                      name=f"{nm}{j}_{v}")
                ps_pair[j] = t
                nk = len(blocks)
                for cc in range(2):
                    c = j * 2 + cc
                    dst = t[:, cc * 512:cc * 512 + HS]
                    for k, (st, wn, r0, r1) in enumerate(blocks):
                        nc.tensor.matmul(dst, st[:, bass.ts(c, 128)],
                                         wt[wn][r0:r1, col0:col1],
                                         start=(k == 0), stop=(k == nk - 1))

            # r c01, hn c01, r c23, hn c23, then z (consumed late)
            gate_mms(ps_r, 0, rz_blocks, 0, HS, "psr")
            gate_mms(ps_hn, 0, hn_blocks, 0, HS, "pshn")
            gate_mms(ps_r, 1, rz_blocks, 0, HS, "psr")
            gate_mms(ps_hn, 1, hn_blocks, 0, HS, "pshn")
            gate_mms(ps_z, 0, rz_blocks, HS, 2 * HS, "psz")
            gate_mms(ps_z, 1, rz_blocks, HS, 2 * HS, "psz")

            # ---------- GRU elementwise (c-half pipelined) ----------
            # Act priority: r0, r1, tanh0, z0, tanh1, z1
            r_sb = sp.tile([128, FLAT], F16, tag="rsb")
            z_sb = sp.tile([128, FLAT], F16, tag="zsb")
            q_sb = sp.tile([128, FLAT], F16, tag="qsb")
            t_sb = sp.tile([128, FLAT], F16, tag="tsb")
            n_sb = sp.tile([128, FLAT], F16, tag="nsb")
            d_sb = sp.tile([128, FLAT], F16, tag="dsb")
            e_sb = sp.tile([128, FLAT], F16, tag="esb")
            h_sb = sp.tile([128, FLAT], F16, tag="hsb")
            for j in range(2):
                nc.scalar.activation(sb2(r_sb, j), pv2(ps_r[j]), ACTF.Sigmoid)
            for j in range(2):
                nc.vector.tensor_tensor(sb2(q_sb, j), sb2(r_sb, j),
                                        pv2(ps_hn[j]), AL.mult)
                nc.vector.tensor_tensor(sb2(t_sb, j), sb2(q_sb, j),
                                        sb2(in_t, j), AL.add)

            def gru_tail_half(j):
                nc.scalar.activation(sb2(n_sb, j), sb2(t_sb, j), ACTF.Tanh)
                nc.scalar.activation(sb2(z_sb, j), pv2(ps_z[j]), ACTF.Sigmoid)
                nc.vector.tensor_tensor(sb2(d_sb, j), sb2(hinL2, j),
                                        sb2(n_sb, j), AL.subtract)
                nc.vector.tensor_tensor(sb2(e_sb, j), sb2(z_sb, j),
                                        sb2(d_sb, j), AL.mult)
                nc.vector.tensor_tensor(sb2(h_sb, j), sb2(n_sb, j),
                                        sb2(e_sb, j), AL.add)

            # ---------- transpose h -> L1 (j-half pipelined) ----------
            # separate PSUM tiles per half so each half's copies free its
            # buffer independently (pool-ring pressure)
            hT0 = hp.tile([128, BL], F16, tag="ht0")
            hT1 = hp.tile([128, BL], F16, tag="ht1")
            hT2x = (hp.tile([H2ROWS, BL], F16, tag="ht2", name=f"ht2_{v}")
                    if not last else None)
            head2_t = (hp.tile([HDROWS, BL], F16, tag="head2", name="head2")
                       if last else None)
            if not last:
                nc.sync.dma_start(hT2x[45:H2ROWS, :], post.ap()[v])

            def h_transpose_half(j):
                b0, b1 = j * 256, j * 256 + 256
                tr = pp.tile([128, 1024], F16, tag="ps", name=f"trh{j}_{v}")
                for ki, (k0, k1) in enumerate(HT):
                    kw = k1 - k0
                    for cc in range(2):
                        c = 2 * j + cc
                        nc.tensor.matmul(
                            tr[0:kw, ki * 256 + cc * 128: ki * 256 + cc * 128 + 128],
                            h_sb[:, c * HS + k0: c * HS + k1], id_t[:],
                            is_transpose=True, skip_group_check=True)
                # parallel copies: hT0 on Act, hT1 + h2-rows on DVE
                nc.scalar.copy(hT0[:, b0:b1], tr[0:128, 0:256])
                nc.vector.tensor_copy(hT1[:, b0:b1], tr[0:128, 256:512])
                dst45 = hT2x if not last else head2_t
                nc.vector.tensor_copy(dst45[0:45, b0:b1], tr[0:45, 512:768])

            if last:
                nc.vector.memset(head2_t[32:64, :], 0.0)
                for j in range(2):
                    gru_tail_half(j)
                    h_transpose_half(j)
                nc.vector.tensor_copy(head2_t[64:64 + FEAT, :], hdT_t[:])
                nc.sync.dma_start(head2_t[HDROWS - 1:HDROWS, :], ones1.ap()[:])
                head2 = head2_t
                hT0_last, hT1_last = hT0, hT1
                break

            # ---------- gate/mapper -> g_v; agg finalize (per j-half) ------
            ps_gg = [None, None]
            ps_gm = [None, None]
            g_blocks = [(hT0, "w_g0", 0, 128), (hT1, "w_g1", 0, 128),
                        (hT2x, "w_g2", 0, H2ROWS)]
            m_blocks = [(hT0, "w_m0", 0, 128), (hT1, "w_m1", 0, 128),
                        (hT2x, "w_m2", 0, H2ROWS)]
            sg_sb = sp.tile([128, FLAT], F16, tag="sgsb")
            gv = g_tiles[v]
            hinL2_next = hp.tile([128, FLAT], F16, tag="hinl2")
            hinT0 = hp.tile([128, BL], F16, tag="hinT0")
            hinT1 = hp.tile([128, BL], F16, tag="hinT1")

            # fold partials + backbone into the PSUM accumulator, per c
            started = bool(pe_us)
            folds = []
            if dve_part is not None:
                folds.append(dve_part)
            if pool_part is not None:
                folds.append(pool_part)

            def part_folds(j):
                """dve/pool partial id-folds for half j (ready before gv)."""
                for fi, ft in enumerate(folds):
                    for c in (2 * j, 2 * j + 1):
                        dst = agg_ps[j][:, (c % 2) * 512:(c % 2) * 512 + HS]
                        nc.tensor.matmul(
                            dst, id_t[:], ft[:, bass.ts(c, HS)],
                            start=(not started and fi == 0), stop=False)

            def agg_finalize_half(j):
                """Backbone + PSUM fold for c in {2j, 2j+1}."""
                if psum_path:
                    for c in (2 * j, 2 * j + 1):
                        dst = agg_ps[j][:, (c % 2) * 512:(c % 2) * 512 + HS]
                        nc.tensor.matmul(
                            dst, id_t[:], gv[:, bass.ts(c, HS)],
                            start=(not started and not folds),
                            stop=True)
                    fe = fold_engine(FOLD_ENG) if j == 1 else nc.vector
                    if fe is None:
                        nc.scalar.activation(sb2(hinL2_next, j),
                                             pv2(agg_ps[j]), ACTF.Copy)
                    else:
                        fe.tensor_copy(sb2(hinL2_next, j), pv2(agg_ps[j]))
                else:
                    for c in (2 * j, 2 * j + 1):
                        hslab = hinL2_next[:, bass.ts(c, HS)]
                        egsl = gv[:, bass.ts(c, HS)]
                        if not dve1_us:
                            nc.vector.tensor_copy(hslab, egsl)
                        else:
                            for i, u in enumerate(dve1_us):
                                gsl = g_tiles[u][:, bass.ts(c, HS)]
                                sc = adj_t[c][:, w * MAXN + u:
                                              w * MAXN + u + 1]
                                if i == 0:
                                    nc.vector.tensor_scalar(
                                        hslab, gsl, sc, None, AL.mult)
                                else:
                                    nc.vector.scalar_tensor_tensor(
                                        hslab, gsl, sc, hslab, AL.mult, AL.add)
                            nc.vector.tensor_tensor(hslab, egsl, hslab, AL.add)

            def hin_transpose_half(j):
                b0, b1 = j * 256, j * 256 + 256
                tr = pp.tile([128, 1024], F16, tag="ps", name=f"trn{j}_{v}")
                for ki, (k0, k1) in enumerate(HT):
                    kw = k1 - k0
                    for cc in range(2):
                        c = 2 * j + cc
                        nc.tensor.matmul(
                            tr[0:kw, ki * 256 + cc * 128: ki * 256 + cc * 128 + 128],
                            hinL2_next[:, c * HS + k0: c * HS + k1], id_t[:],
                            is_transpose=True, skip_group_check=True)
                # parallel copies: hinT0 on DVE, hinT1 on Act, h2-rows on DVE
                nc.vector.tensor_copy(hinT0[:, b0:b1], tr[0:128, 0:256])
                nc.scalar.copy(hinT1[:, b0:b1], tr[0:128, 256:512])
                nc.vector.tensor_copy(xh2_next[HIN2:HIN2 + 45, b0:b1],
                                      tr[0:45, 512:768])

            # half 0 through the whole tail, then half 1
            gru_tail_half(0)
            h_transpose_half(0)
            gru_tail_half(1)
            gate_mms(ps_gg, 0, g_blocks, 0, HS, "psgg")
            gate_mms(ps_gm, 0, m_blocks, 0, HS, "psgm")
            nc.scalar.activation(sb2(sg_sb, 0), pv2(ps_gg[0]), ACTF.Sigmoid)
            nc.vector.tensor_tensor(sb2(gv, 0), sb2(sg_sb, 0),
                                    pv2(ps_gm[0]), AL.mult)
            h_transpose_half(1)
            gate_mms(ps_gg, 1, g_blocks, 0, HS, "psgg")
            nc.scalar.activation(sb2(sg_sb, 1), pv2(ps_gg[1]), ACTF.Sigmoid)
            # DVE agg chain part 2 fills the gate-matmul window
            if dve2_us:
                for c in range(NBT):
                    dslab = dve_part[:, bass.ts(c, HS)]
                    for i, u in enumerate(dve2_us):
                        gsl = g_tiles[u][:, bass.ts(c, HS)]
                        sc = adj_t[c][:, w * MAXN + u: w * MAXN + u + 1]
                        if not dve1_us and i == 0:
                            nc.vector.tensor_scalar(dslab, gsl, sc, None,
                                                    AL.mult)
                        else:
                            nc.vector.scalar_tensor_tensor(
                                dslab, gsl, sc, dslab, AL.mult, AL.add)
            if psum_path and folds:
                part_folds(0)
                part_folds(1)
            agg_finalize_half(0)
            hin_transpose_half(0)
            # mapper j1 PSUM lives in the agps ring so the pp ring lets the
            # next step's r-matmuls start off the j0 tail
            gate_mms(ps_gm, 1, m_blocks, 0, HS, "psgm", pool=ap_)
            nc.vector.tensor_tensor(sb2(gv, 1), sb2(sg_sb, 1),
                                    pv2(ps_gm[1]), AL.mult)
            agg_finalize_half(1)
            hin_transpose_half(1)

            xh2 = xh2_next
            in_t = in_t_next
            hinL2 = hinL2_next
            dg_cur = dg_next

        # ---------- head ----------
        ps_o = [pp.tile([128, 1024], F32, tag="ps", name=f"pso{j}")
                for j in range(2)]
        hd_blocks = [(hT0_last, "w_hd0"), (hT1_last, "w_hd1"),
                     (head2, "w_hd2")]
        for c in range(NBT):
            dst = ps_o[c // 2][:, (c % 2) * 512:(c % 2) * 512 + 112]
            for k, (st, wn) in enumerate(hd_blocks):
                nc.tensor.matmul(dst, st[:, bass.ts(c, 128)], wt[wn][:],
                                 start=(k == 0), stop=(k == 2))
        out_sb = sp.tile([128, NBT * 112], F32, tag="outsb")
        for j in range(2):
            nc.scalar.activation(
                out_sb[:, j * 224:(j + 1) * 224].rearrange(
                    "p (c w) -> p c w", c=2),
                ps_o[j][:].rearrange("p (c w) -> p c w", c=2)[:, :, 0:112],
                ACTF.Copy)
        nc.sync.dma_start(
            out_d.ap().rearrange("c p w -> p c w"),
            out_sb[:].rearrange("p (c w) -> p c w", c=NBT))
    if CAP_WAITS:
        _cap_sync_waits(nc)
    return nc


def _cap_sync_waits(nc, maxw=2):
    """Walrus codegen in this build supports at most `maxw` sem waits per
    instruction (1 for Drain/NoOp ctrl structs).  Move overflow waits onto
    same-engine NoOp instructions inserted immediately before."""
    fn = nc.m.functions[0]
    nid = [0]
    for bb in fn.blocks:
        insts = list(bb.instructions)
        out = []
        for inst in insts:
            si = inst.sync_info
            waits = list(si.on_wait) if si and si.on_wait else []
            limit = 1
            if len(waits) > limit:
                keep = waits[len(waits) - limit:]
                extra = waits[:len(waits) - limit]
                for w in extra:
                    nop = mybir.InstNoOp(name=f"WCAP-{nid[0]}")
                    nid[0] += 1
                    nop.engine = inst.engine
                    nop.sync_info = mybir.SyncInfo(on_wait=[w], on_update=[])
                    out.append(nop)
                si.on_wait = keep
            out.append(inst)
        bb.instructions = out


def _make_in_maps(inp):
    W = _prep_weights(inp)
    f16 = np.float16

    types_, pos_ = inp["types"], inp["pos"]
    X = np.zeros((B, MAXN, 65), f16)
    X[np.arange(B)[:, None], np.arange(MAXN)[None, :], types_] = 1
    X[np.arange(B)[:, None], np.arange(MAXN)[None, :], NVT + pos_] = 1
    X[:, :, XD] = 1.0    # ones row for rz bias

    pos_oh = np.zeros((B, MAXN, 10), f16)
    pos_oh[np.arange(B)[:, None], np.arange(MAXN)[None, :], pos_] = 1
    pos_oh[:, :, 9] = 1.0  # ones row (gate bias)

    # host-gathered i_n = W_ihn[:, type] + W_ihn[:, 26+pos] + b_ihn, as a
    # [234, 301] combined table indexed by type*9+pos
    Wih_n = inp["W_ih"][2 * HS:3 * HS]                        # [301, 35]
    b_ihn = inp["b_ih"][2 * HS:3 * HS]
    combo = (Wih_n[:, :NVT][:, :, None] + Wih_n[:, NVT:][:, None, :]
             + b_ihn[:, None, None])                          # [301, 26, 9]
    combo = combo.reshape(HS, NVT * P9)                       # [301, 234]
    idx = (np.asarray(types_) * P9 + np.asarray(pos_))        # [B, MAXN]
    in_full = combo.T[idx]                                    # [B, MAXN, 301]

    adjf = inp["adj"].astype(np.float32)
    hdf_ = inp["hdf"].astype(np.float32)
    terms, step_off, step_cnt = _pe_terms()

    in_maps = []
    ar = np.arange(128)
    for core in range(NCORES):
        sl = slice(core * BL, (core + 1) * BL)
        m = {}
        m["xt"] = np.ascontiguousarray(X[sl].transpose(1, 2, 0))
        m["post"] = np.ascontiguousarray(pos_oh[sl].transpose(1, 2, 0))
        # int_[v, p, c*301:] = i_n for batch row (c*128 + p)
        inc = in_full[sl].reshape(NBT, 128, MAXN, HS)
        m["int_"] = np.ascontiguousarray(
            inc.transpose(2, 1, 0, 3).reshape(MAXN, 128, FLAT).astype(f16))
        m["ones1"] = np.ones((1, BL), f16)
        m["adjt"] = np.ascontiguousarray(adjf[sl].reshape(NBT, 128, MAXN * MAXN))
        adj_core = adjf[sl].reshape(NBT, 128, MAXN, MAXN)
        npe4 = max(len(terms), 1)
        dg = np.zeros((128, npe4, 128), f16)
        for k, (wv, u, c) in enumerate(terms):
            dg[ar, k, ar] = adj_core[c, :, wv, u]
        m["diagall"] = np.ascontiguousarray(dg.reshape(128, npe4 * 128))
        hd = np.zeros((28, BL), f16)
        hd[0:27] = hdf_[sl].T.astype(f16)
        hd[27] = 1.0
        m["hdft"] = hd
        m["ident"] = np.eye(128, dtype=f16)
        for k, v in W.items():
            m[k] = np.ascontiguousarray(v)
        in_maps.append(m)
    return in_maps


_CACHE = {}


def _get_nc():
    _patch_tile_drain()
    if "nc" not in _CACHE:
        nc = bass.Bass("TRN2", target_bir_lowering=False, debug=False)
        _build(nc)
        _CACHE["nc"] = nc
    return _CACHE["nc"]


def kernel(types, pos, adj, hdf, Wg, bg, Wm, W_ih, W_hh, b_ih, b_hh,
           Wd1, bd1, Wd2, bd2, Wmu, bmu, Wlv, blv, _return_nc=False):
    inp = dict(types=types, pos=pos, adj=adj, hdf=hdf, Wg=Wg, bg=bg, Wm=Wm,
               W_ih=W_ih, W_hh=W_hh, b_ih=b_ih, b_hh=b_hh, Wd1=Wd1, bd1=bd1,
               Wd2=Wd2, bd2=bd2, Wmu=Wmu, bmu=bmu, Wlv=Wlv, blv=blv)
    inp = {k: np.asarray(v) for k, v in inp.items()}
    in_maps = _make_in_maps(inp)
    nc = _get_nc()

    res = run_bass_kernel_spmd(nc, in_maps, list(range(NCORES)))
    mu = np.zeros((B, NZ), np.float32)
    lv = np.zeros((B, NZ), np.float32)
    for core in range(NCORES):
        o = res.results[core]["out"].reshape(BL, 112)
        sl = slice(core * BL, (core + 1) * BL)
        mu[sl] = o[:, 0:NZ]
        lv[sl] = o[:, NZ:112]
    if _return_nc:
        return (mu, lv), res
    return mu, lv


# revision 64
# speedup vs baseline: 1.0608x; 1.0608x over previous
"""CktGNN encoder kernel for Trainium2 (Bass/Tile), 8-core data parallel.

Per core (local batch BL=512 = 4 b-tiles of 128):
  - "L2" tensors: [128 b-partitions, 4*HS free] fp16 (r/z/n/h/Hin/G).
  - "L1" tensors: [hs-partitions, 512 b free] fp16 (transposed h/Hin used as
    matmul stationary operands; produced by PE transpose each step).
  - All matmuls fp16 (1 cyc/row on PE), fp32 PSUM accumulation.
  - Biases folded into matmuls via ones-rows in the stationary data stack.
  - Aggregation Hin_w = sum_u adj[:,w,u] * g_u split across four mechanisms:
      * PE diagonal-matmul accumulation (host-prebuilt diag(adj) streamed
        from HBM in ONE batched DMA per step),
      * DVE scalar_tensor_tensor chains into a partial tile,
      * GPSIMD (Pool) stt chains into a second partial tile,
      * partials + backbone edge (u=w-1, always 1) folded into the PSUM
        accumulator with identity matmuls; the final PSUM->SBUF fold runs on
        a configurable engine (default Pool).
"""
import sys
sys.path.insert(0, "/opt/trn_rl_repo")

import numpy as np
import concourse.bass as bass
import concourse.tile as tile
from concourse import mybir
from concourse.bass_utils import run_bass_kernel_spmd
from concourse.vector_clock import ScopedClock
from contextlib import ExitStack

F16 = mybir.dt.float16
F32 = mybir.dt.float32
AL = mybir.AluOpType
ACTF = mybir.ActivationFunctionType

B = 4096
NCORES = 8
BL = B // NCORES          # 512
NBT = BL // 128           # 4 b-tiles
MAXN = 32
NVT = 26
P9 = 9
XD = NVT + P9             # 35
HS = 301
EMB = 16
FEAT = 8
NZ = 56
FLAT = NBT * HS           # 1204
HALF = 2 * HS             # 602

# xh2 tile rows: [X(35); ones(35); zero pad 36:64; HinT2 64:109 (45);
# ones(109)].  HinT2 sits at 64 so the DVE copy/memset and the hn matmul
# K-block start at a legal base partition (0/32/64/96).
XROWS = 110
HIN2 = 64                 # row offset of HinT2 block in xh2

# hT2x tile rows: [hT2(45); pos 45:54 (9); ones(54)]
H2ROWS = 55
# head2 rows: [hT2(45); pad 45:64; Hd 64:72 (8); ones(72)]
HDROWS = 73

# hs tiling for transposes
HT = [(0, 128), (128, 256), (256, 301)]

# ---- masked-agg term split (tunable) ----
DVE_CAP = 4               # max pairs per step on DVE stt chains
POOL_CAP = 6              # max pairs per step on GPSIMD stt chains
DVE_CAP_LATE = 6          # caps for late steps (2-step-deep chain hoisting)
POOL_CAP_LATE = 10
LATE_W = 16
PURE_DVE_MAX = 2          # steps with <= this many masked terms skip PSUM agg
DVE_P1_FRAC = 0.7         # share of DVE terms emitted early (part 1)

CAP_WAITS = True          # split >1 sem waits onto NoOps (walrus quirk)
FOLD_ENG = "act"          # agg psum -> hinL2 fold: "pool" | "act" | "dve"

_patched = [False]


def _patch_tile_drain():
    """This walrus build only supports ONE sem wait on a Drain instruction.
    Split the kernel-tail drain's waits across several drains."""
    if _patched[0]:
        return
    _patched[0] = True

    def patched(self, tick_clock, wait_clock):
        drain_inst = self.nc.sync.drain()
        wait_clock.add_sem_waits(
            drain_inst.ins, ScopedClock({None: tick_clock.global_clock})
        )
        si = drain_inst.ins.sync_info
        waits = list(si.on_wait or [])
        if len(waits) > 1:
            si.on_wait = waits[:1]
            for w in waits[1:]:
                d2 = self.nc.sync.drain()
                si2 = d2.ins.sync_info
                if si2 is None:
                    d2.ins.sync_info = mybir.SyncInfo(on_wait=[w], on_update=[])
                else:
                    si2.on_wait = [w]
        self.nc.all_engine_barrier()
        popped = self.nc._tile_sem_poison_stack.pop()
        assert popped is self._sem_poison
        self.nc.clear_and_free_semaphores(list(self.sems.allocated().values()))
        self.nc.all_engine_barrier()

    tile.TileContext._drain_and_barrier = patched


def _assign(w):
    """Split masked agg terms u in [0, w-2] for target vertex w.
    Returns (pe_us, pool_us, dve1_us, dve2_us).
    The backbone edge u = w-1 is handled separately."""
    us = list(range(w - 1))
    n = len(us)
    if n <= PURE_DVE_MAX:
        return [], [], us, []
    dcap = DVE_CAP_LATE if w >= LATE_W else DVE_CAP
    pcap = POOL_CAP_LATE if w >= LATE_W else POOL_CAP
    ndve = min(dcap, max(n - pcap - 1, 0))
    npool = min(pcap, n - ndve)
    pe_us = us[:n - ndve - npool]
    pool_us = us[n - ndve - npool:n - ndve]
    dve_us = us[n - ndve:]
    nd1 = int(round(DVE_P1_FRAC * len(dve_us)))
    return pe_us, pool_us, dve_us[:nd1], dve_us[nd1:]


def _pe_terms():
    """Flat ordering of (w, u, c) for the host-built diag tensor, grouped by
    step.  Returns (terms, step_off, step_cnt)."""
    terms = []
    step_off = {}
    step_cnt = {}
    for w in range(2, MAXN):
        pe_us, _, _, _ = _assign(w)
        step_off[w] = len(terms)
        for u in pe_us:
            for c in range(NBT):
                terms.append((w, u, c))
        step_cnt[w] = len(terms) - step_off[w]
    return terms, step_off, step_cnt


def _prep_weights(inp):
    f16 = np.float16
    W = {}
    Wg, bg, Wm = inp["Wg"], inp["bg"], inp["Wm"]
    W_ih, W_hh = inp["W_ih"], inp["W_hh"]
    b_ih, b_hh = inp["b_ih"], inp["b_hh"]

    # gate/mapper moving operands [K, 301]; K-split matches hT0/hT1/hT2x
    # hT2x rows: [h 256:301 (45); pos (9); ones (1)] = 55
    W["w_g0"] = Wg[0:128].astype(f16)
    W["w_g1"] = Wg[128:256].astype(f16)
    W["w_g2"] = np.vstack([Wg[256:301], Wg[301:310], bg[None, :]]).astype(f16)
    W["w_m0"] = Wm[0:128].astype(f16)
    W["w_m1"] = Wm[128:256].astype(f16)
    W["w_m2"] = np.vstack(
        [Wm[256:301], Wm[301:310], np.zeros((1, HS), np.float32)]
    ).astype(f16)

    # r,z: rows 0:602.  K-block 0 matches xh2 rows (110):
    # [X(35); ones(35); pad 36:64; HinT2 64:109; ones(109, zero weight)]
    Wih_rz = W_ih[0:2 * HS]
    Whh_rz = W_hh[0:2 * HS]
    b_rz = (b_ih + b_hh)[0:2 * HS]
    z28 = np.zeros((28, 2 * HS), np.float32)
    W["w_rz0"] = np.vstack([
        Wih_rz.T, b_rz[None, :], z28,
        Whh_rz[:, 256:301].T, np.zeros((1, 2 * HS), np.float32),
    ]).astype(f16)                                   # [110, 602]
    W["w_rz1"] = Whh_rz[:, 0:128].T.astype(f16)
    W["w_rz2"] = Whh_rz[:, 128:256].T.astype(f16)

    # hn: rows 602:903.  K-blocks: HinT0, HinT1, xh2[64:110]=[HinT2; ones]
    Whh_n = W_hh[2 * HS:3 * HS]
    b_hhn = b_hh[2 * HS:3 * HS]
    W["w_hn0"] = Whh_n[:, 0:128].T.astype(f16)
    W["w_hn1"] = Whh_n[:, 128:256].T.astype(f16)
    W["w_hnx"] = np.vstack([
        np.zeros((64, HS), np.float32), Whh_n[:, 256:301].T, b_hhn[None, :]
    ]).astype(f16)                                   # [110, 301]; rows 64:110

    # df encoder
    W["w_d1"] = np.vstack([inp["Wd1"], inp["bd1"][None, :]]).astype(f16)  # [28,16]
    W["w_d2"] = np.vstack([inp["Wd2"], inp["bd2"][None, :]]).astype(f16)  # [17,8]

    # head over head2 rows: [h 256:301 (45); pad 19; Hd (8); ones (1)] = 73
    Whead = np.concatenate([inp["Wmu"], inp["Wlv"]], 1)   # [309, 112]
    bhead = np.concatenate([inp["bmu"], inp["blv"]])
    W["w_hd0"] = Whead[0:128].astype(f16)
    W["w_hd1"] = Whead[128:256].astype(f16)
    W["w_hd2"] = np.vstack([
        Whead[256:301], np.zeros((19, 112), np.float32),
        Whead[301:309], bhead[None, :]
    ]).astype(f16)                                   # [73, 112]
    return W


def _build(nc):
    din = {}

    def dram(name, shape, dt, out=False):
        t = nc.dram_tensor(name, list(shape), dt,
                           kind="ExternalOutput" if out else "ExternalInput")
        din[name] = t
        return t

    terms, step_off, step_cnt = _pe_terms()
    npe4 = max(len(terms), 1)
    xt = dram("xt", [MAXN, 65, BL], F16)            # X rows + ones(35) + pad
    post = dram("post", [MAXN, 10, BL], F16)        # pos one-hot + ones rows
    int_d = dram("int_", [MAXN, 128, FLAT], F16)    # host-gathered i_n + bias
    ones1 = dram("ones1", [1, BL], F16)
    adjt = dram("adjt", [NBT, 128, MAXN * MAXN], F32)
    diagall = dram("diagall", [128, npe4 * 128], F16)
    hdft = dram("hdft", [28, BL], F16)
    ident = dram("ident", [128, 128], F16)
    wnames = [
        ("w_g0", [128, HS]), ("w_g1", [128, HS]), ("w_g2", [H2ROWS, HS]),
        ("w_m0", [128, HS]), ("w_m1", [128, HS]), ("w_m2", [H2ROWS, HS]),
        ("w_rz0", [XROWS, 2 * HS]), ("w_rz1", [128, 2 * HS]),
        ("w_rz2", [128, 2 * HS]),
        ("w_hn0", [128, HS]), ("w_hn1", [128, HS]), ("w_hnx", [XROWS, HS]),
        ("w_d1", [28, EMB]), ("w_d2", [EMB + 1, FEAT]),
        ("w_hd0", [128, 112]), ("w_hd1", [128, 112]), ("w_hd2", [HDROWS, 112]),
    ]
    for n, s in wnames:
        dram(n, s, F16)
    out_d = dram("out", [NBT, 128, 112], F32, out=True)

    max_cnt = max(list(step_cnt.values()) + [1])

    with tile.TileContext(nc) as tc, ExitStack() as ctx:
        wp = ctx.enter_context(tc.tile_pool(name="w", bufs=1))
        xp = ctx.enter_context(tc.tile_pool(name="x", bufs=3))
        hp = ctx.enter_context(tc.tile_pool(name="h", bufs=2))
        sp = ctx.enter_context(tc.tile_pool(name="s", bufs=1))
        gp_ = ctx.enter_context(tc.tile_pool(name="g", bufs=1))
        dgp = ctx.enter_context(tc.tile_pool(name="dg", bufs=2))
        itp = ctx.enter_context(tc.tile_pool(name="it", bufs=2))
        pp = ctx.enter_context(tc.tile_pool(name="ps", bufs=2, space="PSUM"))
        ap_ = ctx.enter_context(tc.tile_pool(name="agps", bufs=2, space="PSUM"))

        wt = {}
        for n, s in wnames:
            t = wp.tile(list(s), F16, tag=n, name=f"wt_{n}")
            nc.sync.dma_start(t[:], din[n].ap()[:])
            wt[n] = t
        adj_t = []
        for c in range(NBT):
            t = wp.tile([128, MAXN * MAXN], F32, tag=f"adj{c}", name=f"adj{c}")
            nc.sync.dma_start(t[:], adjt.ap()[c])
            adj_t.append(t)
        id_t = wp.tile([128, 128], F16, tag="ident")
        nc.sync.dma_start(id_t[:], ident.ap()[:])
        hdft_t = wp.tile([28, BL], F16, tag="hdft")
        nc.sync.dma_start(hdft_t[:], hdft.ap()[:])

        g_tiles = [gp_.tile([128, FLAT], F16, tag=f"gv{u}", name=f"gv{u}")
                   for u in range(MAXN - 1)]

        # ---- df encoder ----
        ps_d = pp.tile([128, 1024], F32, tag="ps")
        nc.tensor.matmul(ps_d[0:EMB, 0:BL], wt["w_d1"][:], hdft_t[:],
                         start=True, stop=True)
        relu_t = wp.tile([EMB + 1, BL], F16, tag="relu")
        nc.vector.memset(relu_t[:], 1.0)
        nc.scalar.activation(relu_t[0:EMB, :], ps_d[0:EMB, 0:BL], ACTF.Relu)
        ps_d2 = pp.tile([128, 1024], F32, tag="ps")
        nc.tensor.matmul(ps_d2[0:FEAT, 0:BL], wt["w_d2"][:], relu_t[:],
                         start=True, stop=True)
        hdT_t = wp.tile([FEAT, BL], F16, tag="hdT")
        nc.scalar.activation(hdT_t[:], ps_d2[0:FEAT, 0:BL], ACTF.Copy)

        # ---- per-step helpers ----
        def new_xh2(v):
            t = xp.tile([XROWS, BL], F16, tag="xh2")
            nc.sync.dma_start(t[0:65, :], xt.ap()[v])
            nc.sync.dma_start(t[XROWS - 1:XROWS, :], ones1.ap()[:])
            return t

        def new_int(v):
            t = itp.tile([128, FLAT], F16, tag="int", name=f"int{v}")
            nc.sync.dma_start(t[:], int_d.ap()[v])
            return t

        def new_diag(w):
            """DMA this step's diag tiles in one shot (for agg of step w)."""
            cnt = step_cnt.get(w, 0)
            if cnt == 0:
                return None
            t = dgp.tile([128, max_cnt * 128], F16, tag="diag", name=f"dg{w}")
            off = step_off[w]
            nc.sync.dma_start(
                t[:, 0:cnt * 128],
                diagall.ap()[:, off * 128:(off + cnt) * 128])
            return t

        def pv2(t):
            """[128, 2, 301] view of a [128, 1024] psum tile's two slabs."""
            return t[:].rearrange("p (c w) -> p c w", c=2)[:, :, 0:HS]

        def sb2(t, j):
            """[128, 2, 301] view of half j of a dense [128, FLAT] tile."""
            return t[:, j * HALF:(j + 1) * HALF].rearrange(
                "p (c w) -> p c w", c=2)

        def fold_engine(eng):
            return {"pool": nc.gpsimd, "dve": nc.vector}.get(eng)

        xh2 = new_xh2(0)
        nc.vector.memset(xh2[HIN2:XROWS, :], 0.0)   # Hin(0)=0
        nc.sync.dma_start(xh2[XROWS - 1:XROWS, :], ones1.ap()[:])
        in_t = new_int(0)
        dg_cur = new_diag(2)   # diag for agg(2) (none: w=2 is pure-DVE)
        hinT0 = None
        hinT1 = None
        hinL2 = hp.tile([128, FLAT], F16, tag="hinl2")
        nc.vector.memset(hinL2[:], 0.0)

        hT0_last = hT1_last = head2 = None

        for v in range(MAXN):
            last = v == MAXN - 1
            w = v + 1

            # ---------- prefetches ----------
            if not last:
                xh2_next = new_xh2(w)
                in_t_next = new_int(w)
            dg_next = new_diag(w + 1) if not last else None

            if not last:
                pe_us, pool_us, dve1_us, dve2_us = _assign(w)
                psum_path = bool(pe_us or pool_us or dve2_us) or \
                    (len(dve1_us) > PURE_DVE_MAX)
            else:
                pe_us = pool_us = dve1_us = dve2_us = []
                psum_path = False

            # ---------- agg(w): PSUM alloc + early PE diag terms ----------
            if psum_path:
                agg_ps = [ap_.tile([128, 1024], F32, tag="agps",
                                   name=f"agg{j}_{v}") for j in range(2)]
                for ui, u in enumerate(pe_us):
                    for c in range(NBT):
                        dst = agg_ps[c // 2][:, (c % 2) * 512:(c % 2) * 512 + HS]
                        k = ui * NBT + c
                        nc.tensor.matmul(
                            dst, dg_cur[:, k * 128:(k + 1) * 128],
                            g_tiles[u][:, bass.ts(c, HS)],
                            start=(ui == 0), stop=False)
            else:
                agg_ps = None

            # ---------- agg(w): Pool chain (early) ----------
            pool_part = None
            if pool_us:
                pool_part = hp.tile([128, FLAT], F16, tag="poolpart",
                                    name=f"pp{v}")
                for c in range(NBT):
                    pslab = pool_part[:, bass.ts(c, HS)]
                    for i, u in enumerate(pool_us):
                        gsl = g_tiles[u][:, bass.ts(c, HS)]
                        sc = adj_t[c][:, w * MAXN + u: w * MAXN + u + 1]
                        if i == 0:
                            nc.gpsimd.tensor_scalar(pslab, gsl, sc, None,
                                                    AL.mult)
                        else:
                            nc.gpsimd.scalar_tensor_tensor(
                                pslab, gsl, sc, pslab, AL.mult, AL.add)

            # ---------- agg(w): DVE chain part 1 (early) ----------
            dve_part = None
            if psum_path and (dve1_us or dve2_us):
                dve_part = hp.tile([128, FLAT], F16, tag="dvepart",
                                   name=f"dp{v}")
                for c in range(NBT):
                    dslab = dve_part[:, bass.ts(c, HS)]
                    for i, u in enumerate(dve1_us):
                        gsl = g_tiles[u][:, bass.ts(c, HS)]
                        sc = adj_t[c][:, w * MAXN + u: w * MAXN + u + 1]
                        if i == 0:
                            nc.vector.tensor_scalar(dslab, gsl, sc, None,
                                                    AL.mult)
                        else:
                            nc.vector.scalar_tensor_tensor(
                                dslab, gsl, sc, dslab, AL.mult, AL.add)

            # ---------- GRU matmuls (r, hn interleaved; then z) -------
            rz_blocks = [(xh2[0:XROWS, :], "w_rz0", 0, XROWS)]
            hn_blocks = [(xh2[HIN2:XROWS, :], "w_hnx", HIN2, XROWS)]
            if v > 0:
                rz_blocks += [(hinT0[:], "w_rz1", 0, 128),
                              (hinT1[:], "w_rz2", 0, 128)]
                hn_blocks += [(hinT0[:], "w_hn0", 0, 128),
                              (hinT1[:], "w_hn1", 0, 128)]

            ps_r = [None, None]
            ps_hn = [None, None]
            ps_z = [None, None]

            def gate_mms(ps_pair, j, blocks, col0, col1, nm, pool=None):
                t = (pool or pp).tile([128, 1024], F32,
                                      tag="ps" if pool is None else "agps",
                                      name=f"{nm}{j}_{v}")
                ps_pair[j] = t
                nk = len(blocks)
                for cc in range(2):
                    c = j * 2 + cc
                    dst = t[:, cc * 512:cc * 512 + HS]
                    for k, (st, wn, r0, r1) in enumerate(blocks):
                        nc.tensor.matmul(dst, st[:, bass.ts(c, 128)],
                                         wt[wn][r0:r1, col0:col1],
                                         start=(k == 0), stop=(k == nk - 1))

            # r c01, hn c01, r c23, hn c23, then z (consumed late)
            gate_mms(ps_r, 0, rz_blocks, 0, HS, "psr")
            gate_mms(ps_hn, 0, hn_blocks, 0, HS, "pshn")
            gate_mms(ps_r, 1, rz_blocks, 0, HS, "psr")
            gate_mms(ps_hn, 1, hn_blocks, 0, HS, "pshn")
            gate_mms(ps_z, 0, rz_blocks, HS, 2 * HS, "psz")
            gate_mms(ps_z, 1, rz_blocks, HS, 2 * HS, "psz")

            # ---------- GRU elementwise (c-half pipelined) ----------
            # Act priority: r0, r1, tanh0, z0, tanh1, z1
            r_sb = sp.tile([128, FLAT], F16, tag="rsb")
            z_sb = sp.tile([128, FLAT], F16, tag="zsb")
            q_sb = sp.tile([128, FLAT], F16, tag="qsb")
            t_sb = sp.tile([128, FLAT], F16, tag="tsb")
            n_sb = sp.tile([128, FLAT], F16, tag="nsb")
            d_sb = sp.tile([128, FLAT], F16, tag="dsb")
            e_sb = sp.tile([128, FLAT], F16, tag="esb")
            h_sb = sp.tile([128, FLAT], F16, tag="hsb")
            for j in range(2):
                nc.scalar.activation(sb2(r_sb, j), pv2(ps_r[j]), ACTF.Sigmoid)
            for j in range(2):
                nc.vector.tensor_tensor(sb2(q_sb, j), sb2(r_sb, j),
                                        pv2(ps_hn[j]), AL.mult)
                nc.vector.tensor_tensor(sb2(t_sb, j), sb2(q_sb, j),
                                        sb2(in_t, j), AL.add)

            def gru_tail_half(j):
                nc.scalar.activation(sb2(n_sb, j), sb2(t_sb, j), ACTF.Tanh)
                nc.scalar.activation(sb2(z_sb, j), pv2(ps_z[j]), ACTF.Sigmoid)
                nc.vector.tensor_tensor(sb2(d_sb, j), sb2(hinL2, j),
                                        sb2(n_sb, j), AL.subtract)
                nc.vector.tensor_tensor(sb2(e_sb, j), sb2(z_sb, j),
                                        sb2(d_sb, j), AL.mult)
                nc.vector.tensor_tensor(sb2(h_sb, j), sb2(n_sb, j),
                                        sb2(e_sb, j), AL.add)

            # ---------- transpose h -> L1 (j-half pipelined) ----------
            # separate PSUM tiles per half so each half's copies free its
            # buffer independently (pool-ring pressure)
            hT0 = hp.tile([128, BL], F16, tag="ht0")
            hT1 = hp.tile([128, BL], F16, tag="ht1")
            hT2x = (hp.tile([H2ROWS, BL], F16, tag="ht2", name=f"ht2_{v}")
                    if not last else None)
            head2_t = (hp.tile([HDROWS, BL], F16, tag="head2", name="head2")
                       if last else None)
            if not last:
                nc.sync.dma_start(hT2x[45:H2ROWS, :], post.ap()[v])

            def h_transpose_half(j):
                b0, b1 = j * 256, j * 256 + 256
                tr = pp.tile([128, 1024], F16, tag="ps", name=f"trh{j}_{v}")
                for ki, (k0, k1) in enumerate(HT):
                    kw = k1 - k0
                    for cc in range(2):
                        c = 2 * j + cc
                        nc.tensor.matmul(
                            tr[0:kw, ki * 256 + cc * 128: ki * 256 + cc * 128 + 128],
                            h_sb[:, c * HS + k0: c * HS + k1], id_t[:],
                            is_transpose=True, skip_group_check=True)
                # parallel copies: hT0 on Act, hT1 + h2-rows on DVE
                nc.scalar.copy(hT0[:, b0:b1], tr[0:128, 0:256])
                nc.vector.tensor_copy(hT1[:, b0:b1], tr[0:128, 256:512])
                dst45 = hT2x if not last else head2_t
                nc.vector.tensor_copy(dst45[0:45, b0:b1], tr[0:45, 512:768])

            if last:
                nc.vector.memset(head2_t[32:64, :], 0.0)
                for j in range(2):
                    gru_tail_half(j)
                    h_transpose_half(j)
                nc.vector.tensor_copy(head2_t[64:64 + FEAT, :], hdT_t[:])
                nc.sync.dma_start(head2_t[HDROWS - 1:HDROWS, :], ones1.ap()[:])
                head2 = head2_t
                hT0_last, hT1_last = hT0, hT1
                break

            # ---------- gate/mapper -> g_v; agg finalize (per j-half) ------
            ps_gg = [None, None]
            ps_gm = [None, None]
            g_blocks = [(hT0, "w_g0", 0, 128), (hT1, "w_g1", 0, 128),
                        (hT2x, "w_g2", 0, H2ROWS)]
            m_blocks = [(hT0, "w_m0", 0, 128), (hT1, "w_m1", 0, 128),
                        (hT2x, "w_m2", 0, H2ROWS)]
            sg_sb = sp.tile([128, FLAT], F16, tag="sgsb")
            gv = g_tiles[v]
            hinL2_next = hp.tile([128, FLAT], F16, tag="hinl2")
            hinT0 = hp.tile([128, BL], F16, tag="hinT0")
            hinT1 = hp.tile([128, BL], F16, tag="hinT1")

            # fold partials + backbone into the PSUM accumulator, per c
            started = bool(pe_us)
            folds = []
            if dve_part is not None:
                folds.append(dve_part)
            if pool_part is not None:
                folds.append(pool_part)

            def part_folds(j):
                """dve/pool partial id-folds for half j (ready before gv)."""
                for fi, ft in enumerate(folds):
                    for c in (2 * j, 2 * j + 1):
                        dst = agg_ps[j][:, (c % 2) * 512:(c % 2) * 512 + HS]
                        nc.tensor.matmul(
                            dst, id_t[:], ft[:, bass.ts(c, HS)],
                            start=(not started and fi == 0), stop=False)

            def agg_finalize_half(j):
                """Backbone + PSUM fold for c in {2j, 2j+1}."""
                if psum_path:
                    for c in (2 * j, 2 * j + 1):
                        dst = agg_ps[j][:, (c % 2) * 512:(c % 2) * 512 + HS]
                        nc.tensor.matmul(
                            dst, id_t[:], gv[:, bass.ts(c, HS)],
                            start=(not started and not folds),
                            stop=True)
                    fe = fold_engine(FOLD_ENG) if j == 1 else nc.vector
                    if fe is None:
                        nc.scalar.activation(sb2(hinL2_next, j),
                                             pv2(agg_ps[j]), ACTF.Copy)
                    else:
                        fe.tensor_copy(sb2(hinL2_next, j), pv2(agg_ps[j]))
                else:
                    for c in (2 * j, 2 * j + 1):
                        hslab = hinL2_next[:, bass.ts(c, HS)]
                        egsl = gv[:, bass.ts(c, HS)]
                        if not dve1_us:
                            nc.vector.tensor_copy(hslab, egsl)
                        else:
                            for i, u in enumerate(dve1_us):
                                gsl = g_tiles[u][:, bass.ts(c, HS)]
                                sc = adj_t[c][:, w * MAXN + u:
                                              w * MAXN + u + 1]
                                if i == 0:
                                    nc.vector.tensor_scalar(
                                        hslab, gsl, sc, None, AL.mult)
                                else:
                                    nc.vector.scalar_tensor_tensor(
                                        hslab, gsl, sc, hslab, AL.mult, AL.add)
                            nc.vector.tensor_tensor(hslab, egsl, hslab, AL.add)

            def hin_transpose_half(j):
                b0, b1 = j * 256, j * 256 + 256
                tr = pp.tile([128, 1024], F16, tag="ps", name=f"trn{j}_{v}")
                for ki, (k0, k1) in enumerate(HT):
                    kw = k1 - k0
                    for cc in range(2):
                        c = 2 * j + cc
                        nc.tensor.matmul(
                            tr[0:kw, ki * 256 + cc * 128: ki * 256 + cc * 128 + 128],
                            hinL2_next[:, c * HS + k0: c * HS + k1], id_t[:],
                            is_transpose=True, skip_group_check=True)
                # parallel copies: hinT0 on DVE, hinT1 on Act, h2-rows on DVE
                nc.vector.tensor_copy(hinT0[:, b0:b1], tr[0:128, 0:256])
                nc.scalar.copy(hinT1[:, b0:b1], tr[0:128, 256:512])
                nc.vector.tensor_copy(xh2_next[HIN2:HIN2 + 45, b0:b1],
                                      tr[0:45, 512:768])

            # half 0 through the whole tail, then half 1
            gru_tail_half(0)
            h_transpose_half(0)
            gru_tail_half(1)
            gate_mms(ps_gg, 0, g_blocks, 0, HS, "psgg")
            gate_mms(ps_gm, 0, m_blocks, 0, HS, "psgm")
            nc.scalar.activation(sb2(sg_sb, 0), pv2(ps_gg[0]), ACTF.Sigmoid)
            nc.vector.tensor_tensor(sb2(gv, 0), sb2(sg_sb, 0),
                                    pv2(ps_gm[0]), AL.mult)
            h_transpose_half(1)
            gate_mms(ps_gg, 1, g_blocks, 0, HS, "psgg")
            nc.scalar.activation(sb2(sg_sb, 1), pv2(ps_gg[1]), ACTF.Sigmoid)
            # DVE agg chain part 2 fills the gate-matmul window
            if dve2_us:
                for c in range(NBT):
                    dslab = dve_part[:, bass.ts(c, HS)]
                    for i, u in enumerate(dve2_us):
                        gsl = g_tiles[u][:, bass.ts(c, HS)]
                        sc = adj_t[c][:, w * MAXN + u: w * MAXN + u + 1]
                        if not dve1_us and i == 0:
                            nc.vector.tensor_scalar(dslab, gsl, sc, None,
                                                    AL.mult)
                        else:
                            nc.vector.scalar_tensor_tensor(
                                dslab, gsl, sc, dslab, AL.mult, AL.add)
            if psum_path and folds:
                part_folds(0)
                part_folds(1)
            agg_finalize_half(0)
            hin_transpose_half(0)
            # mapper j1 PSUM lives in the agps ring so the pp ring lets the
            # next step's r-matmuls start off the j0 tail
            gate_mms(ps_gm, 1, m_blocks, 0, HS, "psgm", pool=ap_)
            nc.vector.tensor_tensor(sb2(gv, 1), sb2(sg_sb, 1),
                                    pv2(ps_gm[1]), AL.mult)
            agg_finalize_half(1)
            hin_transpose_half(1)

            xh2 = xh2_next
            in_t = in_t_next
            hinL2 = hinL2_next
            dg_cur = dg_next

        # ---------- head ----------
        ps_o = [pp.tile([128, 1024], F32, tag="ps", name=f"pso{j}")
                for j in range(2)]
        hd_blocks = [(hT0_last, "w_hd0"), (hT1_last, "w_hd1"),
                     (head2, "w_hd2")]
        for c in range(NBT):
            dst = ps_o[c // 2][:, (c % 2) * 512:(c % 2) * 512 + 112]
            for k, (st, wn) in enumerate(hd_blocks):
                nc.tensor.matmul(dst, st[:, bass.ts(c, 128)], wt[wn][:],
                                 start=(k == 0), stop=(k == 2))
        out_sb = sp.tile([128, NBT * 112], F32, tag="outsb")
        for j in range(2):
            nc.scalar.activation(
                out_sb[:, j * 224:(j + 1) * 224].rearrange(
                    "p (c w) -> p c w", c=2),
                ps_o[j][:].rearrange("p (c w) -> p c w", c=2)[:, :, 0:112],
                ACTF.Copy)
        nc.sync.dma_start(
            out_d.ap().rearrange("c p w -> p c w"),
            out_sb[:].rearrange("p (c w) -> p c w", c=NBT))
    if CAP_WAITS:
        _cap_sync_waits(nc)
    return nc


def _cap_sync_waits(nc, maxw=2):
    """Walrus codegen in this build supports at most `maxw` sem waits per
    instruction (1 for Drain/NoOp ctrl structs).  Move overflow waits onto
    same-engine NoOp instructions inserted immediately before."""
    fn = nc.m.functions[0]
    nid = [0]
    for bb in fn.blocks:
        insts = list(bb.instructions)
        out = []
        for inst in insts:
            si = inst.sync_info
            waits = list(si.on_wait) if si and si.on_wait else []
            limit = 1
            if len(waits) > limit:
                keep = waits[len(waits) - limit:]
                extra = waits[:len(waits) - limit]
                for w in extra:
                    nop = mybir.InstNoOp(name=f"WCAP-{nid[0]}")
                    nid[0] += 1
                    nop.engine = inst.engine
                    nop.sync_info = mybir.SyncInfo(on_wait=[w], on_update=[])
                    out.append(nop)
                si.on_wait = keep
            out.append(inst)
        bb.instructions = out


def _make_in_maps(inp):
    W = _prep_weights(inp)
    f16 = np.float16

    types_, pos_ = inp["types"], inp["pos"]
    X = np.zeros((B, MAXN, 65), f16)
    X[np.arange(B)[:, None], np.arange(MAXN)[None, :], types_] = 1
    X[np.arange(B)[:, None], np.arange(MAXN)[None, :], NVT + pos_] = 1
    X[:, :, XD] = 1.0    # ones row for rz bias

    pos_oh = np.zeros((B, MAXN, 10), f16)
    pos_oh[np.arange(B)[:, None], np.arange(MAXN)[None, :], pos_] = 1
    pos_oh[:, :, 9] = 1.0  # ones row (gate bias)

    # host-gathered i_n = W_ihn[:, type] + W_ihn[:, 26+pos] + b_ihn, as a
    # [234, 301] combined table indexed by type*9+pos
    Wih_n = inp["W_ih"][2 * HS:3 * HS]                        # [301, 35]
    b_ihn = inp["b_ih"][2 * HS:3 * HS]
    combo = (Wih_n[:, :NVT][:, :, None] + Wih_n[:, NVT:][:, None, :]
             + b_ihn[:, None, None])                          # [301, 26, 9]
    combo = combo.reshape(HS, NVT * P9)                       # [301, 234]
    idx = (np.asarray(types_) * P9 + np.asarray(pos_))        # [B, MAXN]
    in_full = combo.T[idx]                                    # [B, MAXN, 301]

    adjf = inp["adj"].astype(np.float32)
    hdf_ = inp["hdf"].astype(np.float32)
    terms, step_off, step_cnt = _pe_terms()

    in_maps = []
    ar = np.arange(128)
    for core in range(NCORES):
        sl = slice(core * BL, (core + 1) * BL)
        m = {}
        m["xt"] = np.ascontiguousarray(X[sl].transpose(1, 2, 0))
        m["post"] = np.ascontiguousarray(pos_oh[sl].transpose(1, 2, 0))
        # int_[v, p, c*301:] = i_n for batch row (c*128 + p)
        inc = in_full[sl].reshape(NBT, 128, MAXN, HS)
        m["int_"] = np.ascontiguousarray(
            inc.transpose(2, 1, 0, 3).reshape(MAXN, 128, FLAT).astype(f16))
        m["ones1"] = np.ones((1, BL), f16)
        m["adjt"] = np.ascontiguousarray(adjf[sl].reshape(NBT, 128, MAXN * MAXN))
        adj_core = adjf[sl].reshape(NBT, 128, MAXN, MAXN)
        npe4 = max(len(terms), 1)
        dg = np.zeros((128, npe4, 128), f16)
        for k, (wv, u, c) in enumerate(terms):
            dg[ar, k, ar] = adj_core[c, :, wv, u]
        m["diagall"] = np.ascontiguousarray(dg.reshape(128, npe4 * 128))
        hd = np.zeros((28, BL), f16)
        hd[0:27] = hdf_[sl].T.astype(f16)
        hd[27] = 1.0
        m["hdft"] = hd
        m["ident"] = np.eye(128, dtype=f16)
        for k, v in W.items():
            m[k] = np.ascontiguousarray(v)
        in_maps.append(m)
    return in_maps


_CACHE = {}


def _get_nc():
    _patch_tile_drain()
    if "nc" not in _CACHE:
        nc = bass.Bass("TRN2", target_bir_lowering=False, debug=False)
        _build(nc)
        _CACHE["nc"] = nc
    return _CACHE["nc"]


def kernel(types, pos, adj, hdf, Wg, bg, Wm, W_ih, W_hh, b_ih, b_hh,
           Wd1, bd1, Wd2, bd2, Wmu, bmu, Wlv, blv, _return_nc=False):
    inp = dict(types=types, pos=pos, adj=adj, hdf=hdf, Wg=Wg, bg=bg, Wm=Wm,
               W_ih=W_ih, W_hh=W_hh, b_ih=b_ih, b_hh=b_hh, Wd1=Wd1, bd1=bd1,
               Wd2=Wd2, bd2=bd2, Wmu=Wmu, bmu=bmu, Wlv=Wlv, blv=blv)
    inp = {k: np.asarray(v) for k, v in inp.items()}
    in_maps = _make_in_maps(inp)
    nc = _get_nc()

    res = run_bass_kernel_spmd(nc, in_maps, list(range(NCORES)))
    mu = np.zeros((B, NZ), np.float32)
    lv = np.zeros((B, NZ), np.float32)
    for core in range(NCORES):
        o = res.results[core]["out"].reshape(BL, 112)
        sl = slice(core * BL, (core + 1) * BL)
        mu[sl] = o[:, 0:NZ]
        lv[sl] = o[:, NZ:112]
    if _return_nc:
        return (mu, lv), res
    return mu, lv


# revision 65
# speedup vs baseline: 1.0637x; 1.0027x over previous
"""CktGNN encoder kernel for Trainium2 (Bass/Tile), 8-core data parallel.

Per core (local batch BL=512 = 4 b-tiles of 128):
  - "L2" tensors: [128 b-partitions, 4*HS free] fp16 (r/z/n/h/Hin/G).
  - "L1" tensors: [hs-partitions, 512 b free] fp16 (transposed h/Hin used as
    matmul stationary operands; produced by PE transpose each step).
  - All matmuls fp16 (1 cyc/row on PE), fp32 PSUM accumulation.
  - Biases folded into matmuls via ones-rows in the stationary data stack.
  - Aggregation Hin_w = sum_u adj[:,w,u] * g_u split across four mechanisms:
      * PE diagonal-matmul accumulation (host-prebuilt diag(adj) streamed
        from HBM in ONE batched DMA per step),
      * DVE scalar_tensor_tensor chains into a partial tile,
      * GPSIMD (Pool) stt chains into a second partial tile,
      * partials + backbone edge (u=w-1, always 1) folded into the PSUM
        accumulator with identity matmuls; the final PSUM->SBUF fold runs on
        a configurable engine (default Pool).
"""
import sys
sys.path.insert(0, "/opt/trn_rl_repo")

import numpy as np
import concourse.bass as bass
import concourse.tile as tile
from concourse import mybir
from concourse.bass_utils import run_bass_kernel_spmd
from concourse.vector_clock import ScopedClock
from contextlib import ExitStack

F16 = mybir.dt.float16
F32 = mybir.dt.float32
AL = mybir.AluOpType
ACTF = mybir.ActivationFunctionType

B = 4096
NCORES = 8
BL = B // NCORES          # 512
NBT = BL // 128           # 4 b-tiles
MAXN = 32
NVT = 26
P9 = 9
XD = NVT + P9             # 35
HS = 301
EMB = 16
FEAT = 8
NZ = 56
FLAT = NBT * HS           # 1204
HALF = 2 * HS             # 602

# xh2 tile rows: [X(35); ones(35); zero pad 36:64; HinT2 64:109 (45);
# ones(109)].  HinT2 sits at 64 so the DVE copy/memset and the hn matmul
# K-block start at a legal base partition (0/32/64/96).
XROWS = 110
HIN2 = 64                 # row offset of HinT2 block in xh2

# hT2x tile rows: [hT2(45); pos 45:54 (9); ones(54)]
H2ROWS = 55
# head2 rows: [hT2(45); pad 45:64; Hd 64:72 (8); ones(72)]
HDROWS = 73

# hs tiling for transposes
HT = [(0, 128), (128, 256), (256, 301)]

# ---- masked-agg term split (tunable) ----
DVE_CAP = 4               # max pairs per step on DVE stt chains
POOL_CAP = 7              # max pairs per step on GPSIMD stt chains
DVE_CAP_LATE = 6          # caps for late steps (2-step-deep chain hoisting)
POOL_CAP_LATE = 10
LATE_W = 16
PURE_DVE_MAX = 2          # steps with <= this many masked terms skip PSUM agg
DVE_P1_FRAC = 0.7         # share of DVE terms emitted early (part 1)

CAP_WAITS = True          # split >1 sem waits onto NoOps (walrus quirk)
FOLD_ENG = "act"          # agg psum -> hinL2 fold: "pool" | "act" | "dve"

_patched = [False]


def _patch_tile_drain():
    """This walrus build only supports ONE sem wait on a Drain instruction.
    Split the kernel-tail drain's waits across several drains."""
    if _patched[0]:
        return
    _patched[0] = True

    def patched(self, tick_clock, wait_clock):
        drain_inst = self.nc.sync.drain()
        wait_clock.add_sem_waits(
            drain_inst.ins, ScopedClock({None: tick_clock.global_clock})
        )
        si = drain_inst.ins.sync_info
        waits = list(si.on_wait or [])
        if len(waits) > 1:
            si.on_wait = waits[:1]
            for w in waits[1:]:
                d2 = self.nc.sync.drain()
                si2 = d2.ins.sync_info
                if si2 is None:
                    d2.ins.sync_info = mybir.SyncInfo(on_wait=[w], on_update=[])
                else:
                    si2.on_wait = [w]
        self.nc.all_engine_barrier()
        popped = self.nc._tile_sem_poison_stack.pop()
        assert popped is self._sem_poison
        self.nc.clear_and_free_semaphores(list(self.sems.allocated().values()))
        self.nc.all_engine_barrier()

    tile.TileContext._drain_and_barrier = patched


def _assign(w):
    """Split masked agg terms u in [0, w-2] for target vertex w.
    Returns (pe_us, pool_us, dve1_us, dve2_us).
    The backbone edge u = w-1 is handled separately."""
    us = list(range(w - 1))
    n = len(us)
    if n <= PURE_DVE_MAX:
        return [], [], us, []
    dcap = DVE_CAP_LATE if w >= LATE_W else DVE_CAP
    pcap = POOL_CAP_LATE if w >= LATE_W else POOL_CAP
    ndve = min(dcap, max(n - pcap - 1, 0))
    npool = min(pcap, n - ndve)
    pe_us = us[:n - ndve - npool]
    pool_us = us[n - ndve - npool:n - ndve]
    dve_us = us[n - ndve:]
    nd1 = int(round(DVE_P1_FRAC * len(dve_us)))
    return pe_us, pool_us, dve_us[:nd1], dve_us[nd1:]


def _pe_terms():
    """Flat ordering of (w, u, c) for the host-built diag tensor, grouped by
    step.  Returns (terms, step_off, step_cnt)."""
    terms = []
    step_off = {}
    step_cnt = {}
    for w in range(2, MAXN):
        pe_us, _, _, _ = _assign(w)
        step_off[w] = len(terms)
        for u in pe_us:
            for c in range(NBT):
                terms.append((w, u, c))
        step_cnt[w] = len(terms) - step_off[w]
    return terms, step_off, step_cnt


def _prep_weights(inp):
    f16 = np.float16
    W = {}
    Wg, bg, Wm = inp["Wg"], inp["bg"], inp["Wm"]
    W_ih, W_hh = inp["W_ih"], inp["W_hh"]
    b_ih, b_hh = inp["b_ih"], inp["b_hh"]

    # gate/mapper moving operands [K, 301]; K-split matches hT0/hT1/hT2x
    # hT2x rows: [h 256:301 (45); pos (9); ones (1)] = 55
    W["w_g0"] = Wg[0:128].astype(f16)
    W["w_g1"] = Wg[128:256].astype(f16)
    W["w_g2"] = np.vstack([Wg[256:301], Wg[301:310], bg[None, :]]).astype(f16)
    W["w_m0"] = Wm[0:128].astype(f16)
    W["w_m1"] = Wm[128:256].astype(f16)
    W["w_m2"] = np.vstack(
        [Wm[256:301], Wm[301:310], np.zeros((1, HS), np.float32)]
    ).astype(f16)

    # r,z: rows 0:602.  K-block 0 matches xh2 rows (110):
    # [X(35); ones(35); pad 36:64; HinT2 64:109; ones(109, zero weight)]
    Wih_rz = W_ih[0:2 * HS]
    Whh_rz = W_hh[0:2 * HS]
    b_rz = (b_ih + b_hh)[0:2 * HS]
    z28 = np.zeros((28, 2 * HS), np.float32)
    W["w_rz0"] = np.vstack([
        Wih_rz.T, b_rz[None, :], z28,
        Whh_rz[:, 256:301].T, np.zeros((1, 2 * HS), np.float32),
    ]).astype(f16)                                   # [110, 602]
    W["w_rz1"] = Whh_rz[:, 0:128].T.astype(f16)
    W["w_rz2"] = Whh_rz[:, 128:256].T.astype(f16)

    # hn: rows 602:903.  K-blocks: HinT0, HinT1, xh2[64:110]=[HinT2; ones]
    Whh_n = W_hh[2 * HS:3 * HS]
    b_hhn = b_hh[2 * HS:3 * HS]
    W["w_hn0"] = Whh_n[:, 0:128].T.astype(f16)
    W["w_hn1"] = Whh_n[:, 128:256].T.astype(f16)
    W["w_hnx"] = np.vstack([
        np.zeros((64, HS), np.float32), Whh_n[:, 256:301].T, b_hhn[None, :]
    ]).astype(f16)                                   # [110, 301]; rows 64:110

    # df encoder
    W["w_d1"] = np.vstack([inp["Wd1"], inp["bd1"][None, :]]).astype(f16)  # [28,16]
    W["w_d2"] = np.vstack([inp["Wd2"], inp["bd2"][None, :]]).astype(f16)  # [17,8]

    # head over head2 rows: [h 256:301 (45); pad 19; Hd (8); ones (1)] = 73
    Whead = np.concatenate([inp["Wmu"], inp["Wlv"]], 1)   # [309, 112]
    bhead = np.concatenate([inp["bmu"], inp["blv"]])
    W["w_hd0"] = Whead[0:128].astype(f16)
    W["w_hd1"] = Whead[128:256].astype(f16)
    W["w_hd2"] = np.vstack([
        Whead[256:301], np.zeros((19, 112), np.float32),
        Whead[301:309], bhead[None, :]
    ]).astype(f16)                                   # [73, 112]
    return W


def _build(nc):
    din = {}

    def dram(name, shape, dt, out=False):
        t = nc.dram_tensor(name, list(shape), dt,
                           kind="ExternalOutput" if out else "ExternalInput")
        din[name] = t
        return t

    terms, step_off, step_cnt = _pe_terms()
    npe4 = max(len(terms), 1)
    xt = dram("xt", [MAXN, 65, BL], F16)            # X rows + ones(35) + pad
    post = dram("post", [MAXN, 10, BL], F16)        # pos one-hot + ones rows
    int_d = dram("int_", [MAXN, 128, FLAT], F16)    # host-gathered i_n + bias
    ones1 = dram("ones1", [1, BL], F16)
    adjt = dram("adjt", [NBT, 128, MAXN * MAXN], F32)
    diagall = dram("diagall", [128, npe4 * 128], F16)
    hdft = dram("hdft", [28, BL], F16)
    ident = dram("ident", [128, 128], F16)
    wnames = [
        ("w_g0", [128, HS]), ("w_g1", [128, HS]), ("w_g2", [H2ROWS, HS]),
        ("w_m0", [128, HS]), ("w_m1", [128, HS]), ("w_m2", [H2ROWS, HS]),
        ("w_rz0", [XROWS, 2 * HS]), ("w_rz1", [128, 2 * HS]),
        ("w_rz2", [128, 2 * HS]),
        ("w_hn0", [128, HS]), ("w_hn1", [128, HS]), ("w_hnx", [XROWS, HS]),
        ("w_d1", [28, EMB]), ("w_d2", [EMB + 1, FEAT]),
        ("w_hd0", [128, 112]), ("w_hd1", [128, 112]), ("w_hd2", [HDROWS, 112]),
    ]
    for n, s in wnames:
        dram(n, s, F16)
    out_d = dram("out", [NBT, 128, 112], F32, out=True)

    max_cnt = max(list(step_cnt.values()) + [1])

    with tile.TileContext(nc) as tc, ExitStack() as ctx:
        wp = ctx.enter_context(tc.tile_pool(name="w", bufs=1))
        xp = ctx.enter_context(tc.tile_pool(name="x", bufs=3))
        hp = ctx.enter_context(tc.tile_pool(name="h", bufs=2))
        sp = ctx.enter_context(tc.tile_pool(name="s", bufs=1))
        gp_ = ctx.enter_context(tc.tile_pool(name="g", bufs=1))
        dgp = ctx.enter_context(tc.tile_pool(name="dg", bufs=2))
        itp = ctx.enter_context(tc.tile_pool(name="it", bufs=2))
        pp = ctx.enter_context(tc.tile_pool(name="ps", bufs=2, space="PSUM"))
        ap_ = ctx.enter_context(tc.tile_pool(name="agps", bufs=2, space="PSUM"))

        wt = {}
        for n, s in wnames:
            t = wp.tile(list(s), F16, tag=n, name=f"wt_{n}")
            nc.sync.dma_start(t[:], din[n].ap()[:])
            wt[n] = t
        adj_t = []
        for c in range(NBT):
            t = wp.tile([128, MAXN * MAXN], F32, tag=f"adj{c}", name=f"adj{c}")
            nc.sync.dma_start(t[:], adjt.ap()[c])
            adj_t.append(t)
        id_t = wp.tile([128, 128], F16, tag="ident")
        nc.sync.dma_start(id_t[:], ident.ap()[:])
        hdft_t = wp.tile([28, BL], F16, tag="hdft")
        nc.sync.dma_start(hdft_t[:], hdft.ap()[:])

        g_tiles = [gp_.tile([128, FLAT], F16, tag=f"gv{u}", name=f"gv{u}")
                   for u in range(MAXN - 1)]

        # ---- df encoder ----
        ps_d = pp.tile([128, 1024], F32, tag="ps")
        nc.tensor.matmul(ps_d[0:EMB, 0:BL], wt["w_d1"][:], hdft_t[:],
                         start=True, stop=True)
        relu_t = wp.tile([EMB + 1, BL], F16, tag="relu")
        nc.vector.memset(relu_t[:], 1.0)
        nc.scalar.activation(relu_t[0:EMB, :], ps_d[0:EMB, 0:BL], ACTF.Relu)
        ps_d2 = pp.tile([128, 1024], F32, tag="ps")
        nc.tensor.matmul(ps_d2[0:FEAT, 0:BL], wt["w_d2"][:], relu_t[:],
                         start=True, stop=True)
        hdT_t = wp.tile([FEAT, BL], F16, tag="hdT")
        nc.scalar.activation(hdT_t[:], ps_d2[0:FEAT, 0:BL], ACTF.Copy)

        # ---- per-step helpers ----
        def new_xh2(v):
            t = xp.tile([XROWS, BL], F16, tag="xh2")
            nc.sync.dma_start(t[0:65, :], xt.ap()[v])
            nc.sync.dma_start(t[XROWS - 1:XROWS, :], ones1.ap()[:])
            return t

        def new_int(v):
            t = itp.tile([128, FLAT], F16, tag="int", name=f"int{v}")
            nc.sync.dma_start(t[:], int_d.ap()[v])
            return t

        def new_diag(w):
            """DMA this step's diag tiles in one shot (for agg of step w)."""
            cnt = step_cnt.get(w, 0)
            if cnt == 0:
                return None
            t = dgp.tile([128, max_cnt * 128], F16, tag="diag", name=f"dg{w}")
            off = step_off[w]
            nc.sync.dma_start(
                t[:, 0:cnt * 128],
                diagall.ap()[:, off * 128:(off + cnt) * 128])
            return t

        def pv2(t):
            """[128, 2, 301] view of a [128, 1024] psum tile's two slabs."""
            return t[:].rearrange("p (c w) -> p c w", c=2)[:, :, 0:HS]

        def sb2(t, j):
            """[128, 2, 301] view of half j of a dense [128, FLAT] tile."""
            return t[:, j * HALF:(j + 1) * HALF].rearrange(
                "p (c w) -> p c w", c=2)

        def fold_engine(eng):
            return {"pool": nc.gpsimd, "dve": nc.vector}.get(eng)

        xh2 = new_xh2(0)
        nc.vector.memset(xh2[HIN2:XROWS, :], 0.0)   # Hin(0)=0
        nc.sync.dma_start(xh2[XROWS - 1:XROWS, :], ones1.ap()[:])
        in_t = new_int(0)
        dg_cur = new_diag(2)   # diag for agg(2) (none: w=2 is pure-DVE)
        hinT0 = None
        hinT1 = None
        hinL2 = hp.tile([128, FLAT], F16, tag="hinl2")
        nc.vector.memset(hinL2[:], 0.0)

        hT0_last = hT1_last = head2 = None

        for v in range(MAXN):
            last = v == MAXN - 1
            w = v + 1

            # ---------- prefetches ----------
            if not last:
                xh2_next = new_xh2(w)
                in_t_next = new_int(w)
            dg_next = new_diag(w + 1) if not last else None

            if not last:
                pe_us, pool_us, dve1_us, dve2_us = _assign(w)
                psum_path = bool(pe_us or pool_us or dve2_us) or \
                    (len(dve1_us) > PURE_DVE_MAX)
            else:
                pe_us = pool_us = dve1_us = dve2_us = []
                psum_path = False

            # ---------- agg(w): PSUM alloc + early PE diag terms ----------
            if psum_path:
                agg_ps = [ap_.tile([128, 1024], F32, tag="agps",
                                   name=f"agg{j}_{v}") for j in range(2)]
                for ui, u in enumerate(pe_us):
                    for c in range(NBT):
                        dst = agg_ps[c // 2][:, (c % 2) * 512:(c % 2) * 512 + HS]
                        k = ui * NBT + c
                        nc.tensor.matmul(
                            dst, dg_cur[:, k * 128:(k + 1) * 128],
                            g_tiles[u][:, bass.ts(c, HS)],
                            start=(ui == 0), stop=False)
            else:
                agg_ps = None

            # ---------- agg(w): Pool chain (early) ----------
            pool_part = None
            if pool_us:
                pool_part = hp.tile([128, FLAT], F16, tag="poolpart",
                                    name=f"pp{v}")
                for c in range(NBT):
                    pslab = pool_part[:, bass.ts(c, HS)]
                    for i, u in enumerate(pool_us):
                        gsl = g_tiles[u][:, bass.ts(c, HS)]
                        sc = adj_t[c][:, w * MAXN + u: w * MAXN + u + 1]
                        if i == 0:
                            nc.gpsimd.tensor_scalar(pslab, gsl, sc, None,
                                                    AL.mult)
                        else:
                            nc.gpsimd.scalar_tensor_tensor(
                                pslab, gsl, sc, pslab, AL.mult, AL.add)

            # ---------- agg(w): DVE chain part 1 (early) ----------
            dve_part = None
            if psum_path and (dve1_us or dve2_us):
                dve_part = hp.tile([128, FLAT], F16, tag="dvepart",
                                   name=f"dp{v}")
                for c in range(NBT):
                    dslab = dve_part[:, bass.ts(c, HS)]
                    for i, u in enumerate(dve1_us):
                        gsl = g_tiles[u][:, bass.ts(c, HS)]
                        sc = adj_t[c][:, w * MAXN + u: w * MAXN + u + 1]
                        if i == 0:
                            nc.vector.tensor_scalar(dslab, gsl, sc, None,
                                                    AL.mult)
                        else:
                            nc.vector.scalar_tensor_tensor(
                                dslab, gsl, sc, dslab, AL.mult, AL.add)

            # ---------- GRU matmuls (r, hn interleaved; then z) -------
            rz_blocks = [(xh2[0:XROWS, :], "w_rz0", 0, XROWS)]
            hn_blocks = [(xh2[HIN2:XROWS, :], "w_hnx", HIN2, XROWS)]
            if v > 0:
                rz_blocks += [(hinT0[:], "w_rz1", 0, 128),
                              (hinT1[:], "w_rz2", 0, 128)]
                hn_blocks += [(hinT0[:], "w_hn0", 0, 128),
                              (hinT1[:], "w_hn1", 0, 128)]

            ps_r = [None, None]
            ps_hn = [None, None]
            ps_z = [None, None]

            def gate_mms(ps_pair, j, blocks, col0, col1, nm, pool=None):
                t = (pool or pp).tile([128, 1024], F32,
                                      tag="ps" if pool is None else "agps",
                                      name=f"{nm}{j}_{v}")
                ps_pair[j] = t
                nk = len(blocks)
                for cc in range(2):
                    c = j * 2 + cc
                    dst = t[:, cc * 512:cc * 512 + HS]
                    for k, (st, wn, r0, r1) in enumerate(blocks):
                        nc.tensor.matmul(dst, st[:, bass.ts(c, 128)],
                                         wt[wn][r0:r1, col0:col1],
                                         start=(k == 0), stop=(k == nk - 1))

            # r c01, hn c01, r c23, hn c23, then z (consumed late)
            gate_mms(ps_r, 0, rz_blocks, 0, HS, "psr")
            gate_mms(ps_hn, 0, hn_blocks, 0, HS, "pshn")
            gate_mms(ps_r, 1, rz_blocks, 0, HS, "psr")
            gate_mms(ps_hn, 1, hn_blocks, 0, HS, "pshn")
            gate_mms(ps_z, 0, rz_blocks, HS, 2 * HS, "psz")
            gate_mms(ps_z, 1, rz_blocks, HS, 2 * HS, "psz")

            # ---------- GRU elementwise (c-half pipelined) ----------
            # Act priority: r0, r1, tanh0, z0, tanh1, z1
            r_sb = sp.tile([128, FLAT], F16, tag="rsb")
            z_sb = sp.tile([128, FLAT], F16, tag="zsb")
            q_sb = sp.tile([128, FLAT], F16, tag="qsb")
            t_sb = sp.tile([128, FLAT], F16, tag="tsb")
            n_sb = sp.tile([128, FLAT], F16, tag="nsb")
            d_sb = sp.tile([128, FLAT], F16, tag="dsb")
            e_sb = sp.tile([128, FLAT], F16, tag="esb")
            h_sb = sp.tile([128, FLAT], F16, tag="hsb")
            for j in range(2):
                nc.scalar.activation(sb2(r_sb, j), pv2(ps_r[j]), ACTF.Sigmoid)
            for j in range(2):
                nc.vector.tensor_tensor(sb2(q_sb, j), sb2(r_sb, j),
                                        pv2(ps_hn[j]), AL.mult)
                nc.vector.tensor_tensor(sb2(t_sb, j), sb2(q_sb, j),
                                        sb2(in_t, j), AL.add)

            def gru_tail_half(j):
                nc.scalar.activation(sb2(n_sb, j), sb2(t_sb, j), ACTF.Tanh)
                nc.scalar.activation(sb2(z_sb, j), pv2(ps_z[j]), ACTF.Sigmoid)
                nc.vector.tensor_tensor(sb2(d_sb, j), sb2(hinL2, j),
                                        sb2(n_sb, j), AL.subtract)
                nc.vector.tensor_tensor(sb2(e_sb, j), sb2(z_sb, j),
                                        sb2(d_sb, j), AL.mult)
                nc.vector.tensor_tensor(sb2(h_sb, j), sb2(n_sb, j),
                                        sb2(e_sb, j), AL.add)

            # ---------- transpose h -> L1 (j-half pipelined) ----------
            # separate PSUM tiles per half so each half's copies free its
            # buffer independently (pool-ring pressure)
            hT0 = hp.tile([128, BL], F16, tag="ht0")
            hT1 = hp.tile([128, BL], F16, tag="ht1")
            hT2x = (hp.tile([H2ROWS, BL], F16, tag="ht2", name=f"ht2_{v}")
                    if not last else None)
            head2_t = (hp.tile([HDROWS, BL], F16, tag="head2", name="head2")
                       if last else None)
            if not last:
                nc.sync.dma_start(hT2x[45:H2ROWS, :], post.ap()[v])

            def h_transpose_half(j):
                b0, b1 = j * 256, j * 256 + 256
                tr = pp.tile([128, 1024], F16, tag="ps", name=f"trh{j}_{v}")
                for ki, (k0, k1) in enumerate(HT):
                    kw = k1 - k0
                    for cc in range(2):
                        c = 2 * j + cc
                        nc.tensor.matmul(
                            tr[0:kw, ki * 256 + cc * 128: ki * 256 + cc * 128 + 128],
                            h_sb[:, c * HS + k0: c * HS + k1], id_t[:],
                            is_transpose=True, skip_group_check=True)
                # parallel copies: hT0 on Act, hT1 + h2-rows on DVE
                nc.scalar.copy(hT0[:, b0:b1], tr[0:128, 0:256])
                nc.vector.tensor_copy(hT1[:, b0:b1], tr[0:128, 256:512])
                dst45 = hT2x if not last else head2_t
                nc.vector.tensor_copy(dst45[0:45, b0:b1], tr[0:45, 512:768])

            if last:
                nc.vector.memset(head2_t[32:64, :], 0.0)
                for j in range(2):
                    gru_tail_half(j)
                    h_transpose_half(j)
                nc.vector.tensor_copy(head2_t[64:64 + FEAT, :], hdT_t[:])
                nc.sync.dma_start(head2_t[HDROWS - 1:HDROWS, :], ones1.ap()[:])
                head2 = head2_t
                hT0_last, hT1_last = hT0, hT1
                break

            # ---------- gate/mapper -> g_v; agg finalize (per j-half) ------
            ps_gg = [None, None]
            ps_gm = [None, None]
            g_blocks = [(hT0, "w_g0", 0, 128), (hT1, "w_g1", 0, 128),
                        (hT2x, "w_g2", 0, H2ROWS)]
            m_blocks = [(hT0, "w_m0", 0, 128), (hT1, "w_m1", 0, 128),
                        (hT2x, "w_m2", 0, H2ROWS)]
            sg_sb = sp.tile([128, FLAT], F16, tag="sgsb")
            gv = g_tiles[v]
            hinL2_next = hp.tile([128, FLAT], F16, tag="hinl2")
            hinT0 = hp.tile([128, BL], F16, tag="hinT0")
            hinT1 = hp.tile([128, BL], F16, tag="hinT1")

            # fold partials + backbone into the PSUM accumulator, per c
            started = bool(pe_us)
            folds = []
            if dve_part is not None:
                folds.append(dve_part)
            if pool_part is not None:
                folds.append(pool_part)

            def part_folds(j):
                """dve/pool partial id-folds for half j (ready before gv)."""
                for fi, ft in enumerate(folds):
                    for c in (2 * j, 2 * j + 1):
                        dst = agg_ps[j][:, (c % 2) * 512:(c % 2) * 512 + HS]
                        nc.tensor.matmul(
                            dst, id_t[:], ft[:, bass.ts(c, HS)],
                            start=(not started and fi == 0), stop=False)

            def agg_finalize_half(j):
                """Backbone + PSUM fold for c in {2j, 2j+1}."""
                if psum_path:
                    for c in (2 * j, 2 * j + 1):
                        dst = agg_ps[j][:, (c % 2) * 512:(c % 2) * 512 + HS]
                        nc.tensor.matmul(
                            dst, id_t[:], gv[:, bass.ts(c, HS)],
                            start=(not started and not folds),
                            stop=True)
                    fe = fold_engine(FOLD_ENG) if j == 1 else nc.vector
                    if fe is None:
                        nc.scalar.activation(sb2(hinL2_next, j),
                                             pv2(agg_ps[j]), ACTF.Copy)
                    else:
                        fe.tensor_copy(sb2(hinL2_next, j), pv2(agg_ps[j]))
                else:
                    for c in (2 * j, 2 * j + 1):
                        hslab = hinL2_next[:, bass.ts(c, HS)]
                        egsl = gv[:, bass.ts(c, HS)]
                        if not dve1_us:
                            nc.vector.tensor_copy(hslab, egsl)
                        else:
                            for i, u in enumerate(dve1_us):
                                gsl = g_tiles[u][:, bass.ts(c, HS)]
                                sc = adj_t[c][:, w * MAXN + u:
                                              w * MAXN + u + 1]
                                if i == 0:
                                    nc.vector.tensor_scalar(
                                        hslab, gsl, sc, None, AL.mult)
                                else:
                                    nc.vector.scalar_tensor_tensor(
                                        hslab, gsl, sc, hslab, AL.mult, AL.add)
                            nc.vector.tensor_tensor(hslab, egsl, hslab, AL.add)

            def hin_transpose_half(j):
                b0, b1 = j * 256, j * 256 + 256
                tr = pp.tile([128, 1024], F16, tag="ps", name=f"trn{j}_{v}")
                for ki, (k0, k1) in enumerate(HT):
                    kw = k1 - k0
                    for cc in range(2):
                        c = 2 * j + cc
                        nc.tensor.matmul(
                            tr[0:kw, ki * 256 + cc * 128: ki * 256 + cc * 128 + 128],
                            hinL2_next[:, c * HS + k0: c * HS + k1], id_t[:],
                            is_transpose=True, skip_group_check=True)
                # parallel copies: hinT0 on DVE, hinT1 on Act, h2-rows on DVE
                nc.vector.tensor_copy(hinT0[:, b0:b1], tr[0:128, 0:256])
                nc.scalar.copy(hinT1[:, b0:b1], tr[0:128, 256:512])
                nc.vector.tensor_copy(xh2_next[HIN2:HIN2 + 45, b0:b1],
                                      tr[0:45, 512:768])

            # half 0 through the whole tail, then half 1
            gru_tail_half(0)
            h_transpose_half(0)
            gru_tail_half(1)
            gate_mms(ps_gg, 0, g_blocks, 0, HS, "psgg")
            gate_mms(ps_gm, 0, m_blocks, 0, HS, "psgm")
            nc.scalar.activation(sb2(sg_sb, 0), pv2(ps_gg[0]), ACTF.Sigmoid)
            nc.vector.tensor_tensor(sb2(gv, 0), sb2(sg_sb, 0),
                                    pv2(ps_gm[0]), AL.mult)
            h_transpose_half(1)
            gate_mms(ps_gg, 1, g_blocks, 0, HS, "psgg")
            nc.scalar.activation(sb2(sg_sb, 1), pv2(ps_gg[1]), ACTF.Sigmoid)
            # DVE agg chain part 2 fills the gate-matmul window
            if dve2_us:
                for c in range(NBT):
                    dslab = dve_part[:, bass.ts(c, HS)]
                    for i, u in enumerate(dve2_us):
                        gsl = g_tiles[u][:, bass.ts(c, HS)]
                        sc = adj_t[c][:, w * MAXN + u: w * MAXN + u + 1]
                        if not dve1_us and i == 0:
                            nc.vector.tensor_scalar(dslab, gsl, sc, None,
                                                    AL.mult)
                        else:
                            nc.vector.scalar_tensor_tensor(
                                dslab, gsl, sc, dslab, AL.mult, AL.add)
            if psum_path and folds:
                part_folds(0)
                part_folds(1)
            agg_finalize_half(0)
            hin_transpose_half(0)
            # mapper j1 PSUM lives in the agps ring so the pp ring lets the
            # next step's r-matmuls start off the j0 tail
            gate_mms(ps_gm, 1, m_blocks, 0, HS, "psgm", pool=ap_)
            nc.vector.tensor_tensor(sb2(gv, 1), sb2(sg_sb, 1),
                                    pv2(ps_gm[1]), AL.mult)
            agg_finalize_half(1)
            hin_transpose_half(1)

            xh2 = xh2_next
            in_t = in_t_next
            hinL2 = hinL2_next
            dg_cur = dg_next

        # ---------- head ----------
        ps_o = [pp.tile([128, 1024], F32, tag="ps", name=f"pso{j}")
                for j in range(2)]
        hd_blocks = [(hT0_last, "w_hd0"), (hT1_last, "w_hd1"),
                     (head2, "w_hd2")]
        for c in range(NBT):
            dst = ps_o[c // 2][:, (c % 2) * 512:(c % 2) * 512 + 112]
            for k, (st, wn) in enumerate(hd_blocks):
                nc.tensor.matmul(dst, st[:, bass.ts(c, 128)], wt[wn][:],
                                 start=(k == 0), stop=(k == 2))
        out_sb = sp.tile([128, NBT * 112], F32, tag="outsb")
        for j in range(2):
            nc.scalar.activation(
                out_sb[:, j * 224:(j + 1) * 224].rearrange(
                    "p (c w) -> p c w", c=2),
                ps_o[j][:].rearrange("p (c w) -> p c w", c=2)[:, :, 0:112],
                ACTF.Copy)
        nc.sync.dma_start(
            out_d.ap().rearrange("c p w -> p c w"),
            out_sb[:].rearrange("p (c w) -> p c w", c=NBT))
    if CAP_WAITS:
        _cap_sync_waits(nc)
    return nc


def _cap_sync_waits(nc, maxw=2):
    """Walrus codegen in this build supports at most `maxw` sem waits per
    instruction (1 for Drain/NoOp ctrl structs).  Move overflow waits onto
    same-engine NoOp instructions inserted immediately before."""
    fn = nc.m.functions[0]
    nid = [0]
    for bb in fn.blocks:
        insts = list(bb.instructions)
        out = []
        for inst in insts:
            si = inst.sync_info
            waits = list(si.on_wait) if si and si.on_wait else []
            limit = 1
            if len(waits) > limit:
                keep = waits[len(waits) - limit:]
                extra = waits[:len(waits) - limit]
                for w in extra:
                    nop = mybir.InstNoOp(name=f"WCAP-{nid[0]}")
                    nid[0] += 1
                    nop.engine = inst.engine
                    nop.sync_info = mybir.SyncInfo(on_wait=[w], on_update=[])
                    out.append(nop)
                si.on_wait = keep
            out.append(inst)
        bb.instructions = out


def _make_in_maps(inp):
    W = _prep_weights(inp)
    f16 = np.float16

    types_, pos_ = inp["types"], inp["pos"]
    X = np.zeros((B, MAXN, 65), f16)
    X[np.arange(B)[:, None], np.arange(MAXN)[None, :], types_] = 1
    X[np.arange(B)[:, None], np.arange(MAXN)[None, :], NVT + pos_] = 1
    X[:, :, XD] = 1.0    # ones row for rz bias

    pos_oh = np.zeros((B, MAXN, 10), f16)
    pos_oh[np.arange(B)[:, None], np.arange(MAXN)[None, :], pos_] = 1
    pos_oh[:, :, 9] = 1.0  # ones row (gate bias)

    # host-gathered i_n = W_ihn[:, type] + W_ihn[:, 26+pos] + b_ihn, as a
    # [234, 301] combined table indexed by type*9+pos
    Wih_n = inp["W_ih"][2 * HS:3 * HS]                        # [301, 35]
    b_ihn = inp["b_ih"][2 * HS:3 * HS]
    combo = (Wih_n[:, :NVT][:, :, None] + Wih_n[:, NVT:][:, None, :]
             + b_ihn[:, None, None])                          # [301, 26, 9]
    combo = combo.reshape(HS, NVT * P9)                       # [301, 234]
    idx = (np.asarray(types_) * P9 + np.asarray(pos_))        # [B, MAXN]
    in_full = combo.T[idx]                                    # [B, MAXN, 301]

    adjf = inp["adj"].astype(np.float32)
    hdf_ = inp["hdf"].astype(np.float32)
    terms, step_off, step_cnt = _pe_terms()

    in_maps = []
    ar = np.arange(128)
    for core in range(NCORES):
        sl = slice(core * BL, (core + 1) * BL)
        m = {}
        m["xt"] = np.ascontiguousarray(X[sl].transpose(1, 2, 0))
        m["post"] = np.ascontiguousarray(pos_oh[sl].transpose(1, 2, 0))
        # int_[v, p, c*301:] = i_n for batch row (c*128 + p)
        inc = in_full[sl].reshape(NBT, 128, MAXN, HS)
        m["int_"] = np.ascontiguousarray(
            inc.transpose(2, 1, 0, 3).reshape(MAXN, 128, FLAT).astype(f16))
        m["ones1"] = np.ones((1, BL), f16)
        m["adjt"] = np.ascontiguousarray(adjf[sl].reshape(NBT, 128, MAXN * MAXN))
        adj_core = adjf[sl].reshape(NBT, 128, MAXN, MAXN)
        npe4 = max(len(terms), 1)
        dg = np.zeros((128, npe4, 128), f16)
        for k, (wv, u, c) in enumerate(terms):
            dg[ar, k, ar] = adj_core[c, :, wv, u]
        m["diagall"] = np.ascontiguousarray(dg.reshape(128, npe4 * 128))
        hd = np.zeros((28, BL), f16)
        hd[0:27] = hdf_[sl].T.astype(f16)
        hd[27] = 1.0
        m["hdft"] = hd
        m["ident"] = np.eye(128, dtype=f16)
        for k, v in W.items():
            m[k] = np.ascontiguousarray(v)
        in_maps.append(m)
    return in_maps


_CACHE = {}


def _get_nc():
    _patch_tile_drain()
    if "nc" not in _CACHE:
        nc = bass.Bass("TRN2", target_bir_lowering=False, debug=False)
        _build(nc)
        _CACHE["nc"] = nc
    return _CACHE["nc"]


def kernel(types, pos, adj, hdf, Wg, bg, Wm, W_ih, W_hh, b_ih, b_hh,
           Wd1, bd1, Wd2, bd2, Wmu, bmu, Wlv, blv, _return_nc=False):
    inp = dict(types=types, pos=pos, adj=adj, hdf=hdf, Wg=Wg, bg=bg, Wm=Wm,
               W_ih=W_ih, W_hh=W_hh, b_ih=b_ih, b_hh=b_hh, Wd1=Wd1, bd1=bd1,
               Wd2=Wd2, bd2=bd2, Wmu=Wmu, bmu=bmu, Wlv=Wlv, blv=blv)
    inp = {k: np.asarray(v) for k, v in inp.items()}
    in_maps = _make_in_maps(inp)
    nc = _get_nc()

    res = run_bass_kernel_spmd(nc, in_maps, list(range(NCORES)))
    mu = np.zeros((B, NZ), np.float32)
    lv = np.zeros((B, NZ), np.float32)
    for core in range(NCORES):
        o = res.results[core]["out"].reshape(BL, 112)
        sl = slice(core * BL, (core + 1) * BL)
        mu[sl] = o[:, 0:NZ]
        lv[sl] = o[:, NZ:112]
    if _return_nc:
        return (mu, lv), res
    return mu, lv


# revision 66
# speedup vs baseline: 1.0649x; 1.0011x over previous
"""CktGNN encoder kernel for Trainium2 (Bass/Tile), 8-core data parallel.

Per core (local batch BL=512 = 4 b-tiles of 128):
  - "L2" tensors: [128 b-partitions, 4*HS free] fp16 (r/z/n/h/Hin/G).
  - "L1" tensors: [hs-partitions, 512 b free] fp16 (transposed h/Hin used as
    matmul stationary operands; produced by PE transpose each step).
  - All matmuls fp16 (1 cyc/row on PE), fp32 PSUM accumulation.
  - Biases folded into matmuls via ones-rows in the stationary data stack.
  - Aggregation Hin_w = sum_u adj[:,w,u] * g_u split across four mechanisms:
      * PE diagonal-matmul accumulation (host-prebuilt diag(adj) streamed
        from HBM in ONE batched DMA per step),
      * DVE scalar_tensor_tensor chains into a partial tile,
      * GPSIMD (Pool) stt chains into a second partial tile,
      * partials + backbone edge (u=w-1, always 1) folded into the PSUM
        accumulator with identity matmuls; the final PSUM->SBUF fold runs on
        a configurable engine (default Pool).
"""
import sys
sys.path.insert(0, "/opt/trn_rl_repo")

import numpy as np
import concourse.bass as bass
import concourse.tile as tile
from concourse import mybir
from concourse.bass_utils import run_bass_kernel_spmd
from concourse.vector_clock import ScopedClock
from contextlib import ExitStack

F16 = mybir.dt.float16
F32 = mybir.dt.float32
AL = mybir.AluOpType
ACTF = mybir.ActivationFunctionType

B = 4096
NCORES = 8
BL = B // NCORES          # 512
NBT = BL // 128           # 4 b-tiles
MAXN = 32
NVT = 26
P9 = 9
XD = NVT + P9             # 35
HS = 301
EMB = 16
FEAT = 8
NZ = 56
FLAT = NBT * HS           # 1204
HALF = 2 * HS             # 602

# xh2 tile rows: [X(35); ones(35); zero pad 36:64; HinT2 64:109 (45);
# ones(109)].  HinT2 sits at 64 so the DVE copy/memset and the hn matmul
# K-block start at a legal base partition (0/32/64/96).
XROWS = 110
HIN2 = 64                 # row offset of HinT2 block in xh2

# hT2x tile rows: [hT2(45); pos 45:54 (9); ones(54)]
H2ROWS = 55
# head2 rows: [hT2(45); pad 45:64; Hd 64:72 (8); ones(72)]
HDROWS = 73

# hs tiling for transposes
HT = [(0, 128), (128, 256), (256, 301)]

# ---- masked-agg term split (tunable) ----
DVE_CAP = 4               # max pairs per step on DVE stt chains
POOL_CAP = 8              # max pairs per step on GPSIMD stt chains
DVE_CAP_LATE = 6          # caps for late steps (2-step-deep chain hoisting)
POOL_CAP_LATE = 10
LATE_W = 16
PURE_DVE_MAX = 2          # steps with <= this many masked terms skip PSUM agg
DVE_P1_FRAC = 0.7         # share of DVE terms emitted early (part 1)

CAP_WAITS = True          # split >1 sem waits onto NoOps (walrus quirk)
FOLD_ENG = "act"          # agg psum -> hinL2 fold: "pool" | "act" | "dve"

_patched = [False]


def _patch_tile_drain():
    """This walrus build only supports ONE sem wait on a Drain instruction.
    Split the kernel-tail drain's waits across several drains."""
    if _patched[0]:
        return
    _patched[0] = True

    def patched(self, tick_clock, wait_clock):
        drain_inst = self.nc.sync.drain()
        wait_clock.add_sem_waits(
            drain_inst.ins, ScopedClock({None: tick_clock.global_clock})
        )
        si = drain_inst.ins.sync_info
        waits = list(si.on_wait or [])
        if len(waits) > 1:
            si.on_wait = waits[:1]
            for w in waits[1:]:
                d2 = self.nc.sync.drain()
                si2 = d2.ins.sync_info
                if si2 is None:
                    d2.ins.sync_info = mybir.SyncInfo(on_wait=[w], on_update=[])
                else:
                    si2.on_wait = [w]
        self.nc.all_engine_barrier()
        popped = self.nc._tile_sem_poison_stack.pop()
        assert popped is self._sem_poison
        self.nc.clear_and_free_semaphores(list(self.sems.allocated().values()))
        self.nc.all_engine_barrier()

    tile.TileContext._drain_and_barrier = patched


def _assign(w):
    """Split masked agg terms u in [0, w-2] for target vertex w.
    Returns (pe_us, pool_us, dve1_us, dve2_us).
    The backbone edge u = w-1 is handled separately."""
    us = list(range(w - 1))
    n = len(us)
    if n <= PURE_DVE_MAX:
        return [], [], us, []
    dcap = DVE_CAP_LATE if w >= LATE_W else DVE_CAP
    pcap = POOL_CAP_LATE if w >= LATE_W else POOL_CAP
    ndve = min(dcap, max(n - pcap - 1, 0))
    npool = min(pcap, n - ndve)
    pe_us = us[:n - ndve - npool]
    pool_us = us[n - ndve - npool:n - ndve]
    dve_us = us[n - ndve:]
    nd1 = int(round(DVE_P1_FRAC * len(dve_us)))
    return pe_us, pool_us, dve_us[:nd1], dve_us[nd1:]


def _pe_terms():
    """Flat ordering of (w, u, c) for the host-built diag tensor, grouped by
    step.  Returns (terms, step_off, step_cnt)."""
    terms = []
    step_off = {}
    step_cnt = {}
    for w in range(2, MAXN):
        pe_us, _, _, _ = _assign(w)
        step_off[w] = len(terms)
        for u in pe_us:
            for c in range(NBT):
                terms.append((w, u, c))
        step_cnt[w] = len(terms) - step_off[w]
    return terms, step_off, step_cnt


def _prep_weights(inp):
    f16 = np.float16
    W = {}
    Wg, bg, Wm = inp["Wg"], inp["bg"], inp["Wm"]
    W_ih, W_hh = inp["W_ih"], inp["W_hh"]
    b_ih, b_hh = inp["b_ih"], inp["b_hh"]

    # gate/mapper moving operands [K, 301]; K-split matches hT0/hT1/hT2x
    # hT2x rows: [h 256:301 (45); pos (9); ones (1)] = 55
    W["w_g0"] = Wg[0:128].astype(f16)
    W["w_g1"] = Wg[128:256].astype(f16)
    W["w_g2"] = np.vstack([Wg[256:301], Wg[301:310], bg[None, :]]).astype(f16)
    W["w_m0"] = Wm[0:128].astype(f16)
    W["w_m1"] = Wm[128:256].astype(f16)
    W["w_m2"] = np.vstack(
        [Wm[256:301], Wm[301:310], np.zeros((1, HS), np.float32)]
    ).astype(f16)

    # r,z: rows 0:602.  K-block 0 matches xh2 rows (110):
    # [X(35); ones(35); pad 36:64; HinT2 64:109; ones(109, zero weight)]
    Wih_rz = W_ih[0:2 * HS]
    Whh_rz = W_hh[0:2 * HS]
    b_rz = (b_ih + b_hh)[0:2 * HS]
    z28 = np.zeros((28, 2 * HS), np.float32)
    W["w_rz0"] = np.vstack([
        Wih_rz.T, b_rz[None, :], z28,
        Whh_rz[:, 256:301].T, np.zeros((1, 2 * HS), np.float32),
    ]).astype(f16)                                   # [110, 602]
    W["w_rz1"] = Whh_rz[:, 0:128].T.astype(f16)
    W["w_rz2"] = Whh_rz[:, 128:256].T.astype(f16)

    # hn: rows 602:903.  K-blocks: HinT0, HinT1, xh2[64:110]=[HinT2; ones]
    Whh_n = W_hh[2 * HS:3 * HS]
    b_hhn = b_hh[2 * HS:3 * HS]
    W["w_hn0"] = Whh_n[:, 0:128].T.astype(f16)
    W["w_hn1"] = Whh_n[:, 128:256].T.astype(f16)
    W["w_hnx"] = np.vstack([
        np.zeros((64, HS), np.float32), Whh_n[:, 256:301].T, b_hhn[None, :]
    ]).astype(f16)                                   # [110, 301]; rows 64:110

    # df encoder
    W["w_d1"] = np.vstack([inp["Wd1"], inp["bd1"][None, :]]).astype(f16)  # [28,16]
    W["w_d2"] = np.vstack([inp["Wd2"], inp["bd2"][None, :]]).astype(f16)  # [17,8]

    # head over head2 rows: [h 256:301 (45); pad 19; Hd (8); ones (1)] = 73
    Whead = np.concatenate([inp["Wmu"], inp["Wlv"]], 1)   # [309, 112]
    bhead = np.concatenate([inp["bmu"], inp["blv"]])
    W["w_hd0"] = Whead[0:128].astype(f16)
    W["w_hd1"] = Whead[128:256].astype(f16)
    W["w_hd2"] = np.vstack([
        Whead[256:301], np.zeros((19, 112), np.float32),
        Whead[301:309], bhead[None, :]
    ]).astype(f16)                                   # [73, 112]
    return W


def _build(nc):
    din = {}

    def dram(name, shape, dt, out=False):
        t = nc.dram_tensor(name, list(shape), dt,
                           kind="ExternalOutput" if out else "ExternalInput")
        din[name] = t
        return t

    terms, step_off, step_cnt = _pe_terms()
    npe4 = max(len(terms), 1)
    xt = dram("xt", [MAXN, 65, BL], F16)            # X rows + ones(35) + pad
    post = dram("post", [MAXN, 10, BL], F16)        # pos one-hot + ones rows
    int_d = dram("int_", [MAXN, 128, FLAT], F16)    # host-gathered i_n + bias
    ones1 = dram("ones1", [1, BL], F16)
    adjt = dram("adjt", [NBT, 128, MAXN * MAXN], F32)
    diagall = dram("diagall", [128, npe4 * 128], F16)
    hdft = dram("hdft", [28, BL], F16)
    ident = dram("ident", [128, 128], F16)
    wnames = [
        ("w_g0", [128, HS]), ("w_g1", [128, HS]), ("w_g2", [H2ROWS, HS]),
        ("w_m0", [128, HS]), ("w_m1", [128, HS]), ("w_m2", [H2ROWS, HS]),
        ("w_rz0", [XROWS, 2 * HS]), ("w_rz1", [128, 2 * HS]),
        ("w_rz2", [128, 2 * HS]),
        ("w_hn0", [128, HS]), ("w_hn1", [128, HS]), ("w_hnx", [XROWS, HS]),
        ("w_d1", [28, EMB]), ("w_d2", [EMB + 1, FEAT]),
        ("w_hd0", [128, 112]), ("w_hd1", [128, 112]), ("w_hd2", [HDROWS, 112]),
    ]
    for n, s in wnames:
        dram(n, s, F16)
    out_d = dram("out", [NBT, 128, 112], F32, out=True)

    max_cnt = max(list(step_cnt.values()) + [1])

    with tile.TileContext(nc) as tc, ExitStack() as ctx:
        wp = ctx.enter_context(tc.tile_pool(name="w", bufs=1))
        xp = ctx.enter_context(tc.tile_pool(name="x", bufs=3))
        hp = ctx.enter_context(tc.tile_pool(name="h", bufs=2))
        sp = ctx.enter_context(tc.tile_pool(name="s", bufs=1))
        gp_ = ctx.enter_context(tc.tile_pool(name="g", bufs=1))
        dgp = ctx.enter_context(tc.tile_pool(name="dg", bufs=2))
        itp = ctx.enter_context(tc.tile_pool(name="it", bufs=2))
        pp = ctx.enter_context(tc.tile_pool(name="ps", bufs=2, space="PSUM"))
        ap_ = ctx.enter_context(tc.tile_pool(name="agps", bufs=2, space="PSUM"))

        wt = {}
        for n, s in wnames:
            t = wp.tile(list(s), F16, tag=n, name=f"wt_{n}")
            nc.sync.dma_start(t[:], din[n].ap()[:])
            wt[n] = t
        adj_t = []
        for c in range(NBT):
            t = wp.tile([128, MAXN * MAXN], F32, tag=f"adj{c}", name=f"adj{c}")
            nc.sync.dma_start(t[:], adjt.ap()[c])
            adj_t.append(t)
        id_t = wp.tile([128, 128], F16, tag="ident")
        nc.sync.dma_start(id_t[:], ident.ap()[:])
        hdft_t = wp.tile([28, BL], F16, tag="hdft")
        nc.sync.dma_start(hdft_t[:], hdft.ap()[:])

        g_tiles = [gp_.tile([128, FLAT], F16, tag=f"gv{u}", name=f"gv{u}")
                   for u in range(MAXN - 1)]

        # ---- df encoder ----
        ps_d = pp.tile([128, 1024], F32, tag="ps")
        nc.tensor.matmul(ps_d[0:EMB, 0:BL], wt["w_d1"][:], hdft_t[:],
                         start=True, stop=True)
        relu_t = wp.tile([EMB + 1, BL], F16, tag="relu")
        nc.vector.memset(relu_t[:], 1.0)
        nc.scalar.activation(relu_t[0:EMB, :], ps_d[0:EMB, 0:BL], ACTF.Relu)
        ps_d2 = pp.tile([128, 1024], F32, tag="ps")
        nc.tensor.matmul(ps_d2[0:FEAT, 0:BL], wt["w_d2"][:], relu_t[:],
                         start=True, stop=True)
        hdT_t = wp.tile([FEAT, BL], F16, tag="hdT")
        nc.scalar.activation(hdT_t[:], ps_d2[0:FEAT, 0:BL], ACTF.Copy)

        # ---- per-step helpers ----
        def new_xh2(v):
            t = xp.tile([XROWS, BL], F16, tag="xh2")
            nc.sync.dma_start(t[0:65, :], xt.ap()[v])
            nc.sync.dma_start(t[XROWS - 1:XROWS, :], ones1.ap()[:])
            return t

        def new_int(v):
            t = itp.tile([128, FLAT], F16, tag="int", name=f"int{v}")
            nc.sync.dma_start(t[:], int_d.ap()[v])
            return t

        def new_diag(w):
            """DMA this step's diag tiles in one shot (for agg of step w)."""
            cnt = step_cnt.get(w, 0)
            if cnt == 0:
                return None
            t = dgp.tile([128, max_cnt * 128], F16, tag="diag", name=f"dg{w}")
            off = step_off[w]
            nc.sync.dma_start(
                t[:, 0:cnt * 128],
                diagall.ap()[:, off * 128:(off + cnt) * 128])
            return t

        def pv2(t):
            """[128, 2, 301] view of a [128, 1024] psum tile's two slabs."""
            return t[:].rearrange("p (c w) -> p c w", c=2)[:, :, 0:HS]

        def sb2(t, j):
            """[128, 2, 301] view of half j of a dense [128, FLAT] tile."""
            return t[:, j * HALF:(j + 1) * HALF].rearrange(
                "p (c w) -> p c w", c=2)

        def fold_engine(eng):
            return {"pool": nc.gpsimd, "dve": nc.vector}.get(eng)

        xh2 = new_xh2(0)
        nc.vector.memset(xh2[HIN2:XROWS, :], 0.0)   # Hin(0)=0
        nc.sync.dma_start(xh2[XROWS - 1:XROWS, :], ones1.ap()[:])
        in_t = new_int(0)
        dg_cur = new_diag(2)   # diag for agg(2) (none: w=2 is pure-DVE)
        hinT0 = None
        hinT1 = None
        hinL2 = hp.tile([128, FLAT], F16, tag="hinl2")
        nc.vector.memset(hinL2[:], 0.0)

        hT0_last = hT1_last = head2 = None

        for v in range(MAXN):
            last = v == MAXN - 1
            w = v + 1

            # ---------- prefetches ----------
            if not last:
                xh2_next = new_xh2(w)
                in_t_next = new_int(w)
            dg_next = new_diag(w + 1) if not last else None

            if not last:
                pe_us, pool_us, dve1_us, dve2_us = _assign(w)
                psum_path = bool(pe_us or pool_us or dve2_us) or \
                    (len(dve1_us) > PURE_DVE_MAX)
            else:
                pe_us = pool_us = dve1_us = dve2_us = []
                psum_path = False

            # ---------- agg(w): PSUM alloc + early PE diag terms ----------
            if psum_path:
                agg_ps = [ap_.tile([128, 1024], F32, tag="agps",
                                   name=f"agg{j}_{v}") for j in range(2)]
                for ui, u in enumerate(pe_us):
                    for c in range(NBT):
                        dst = agg_ps[c // 2][:, (c % 2) * 512:(c % 2) * 512 + HS]
                        k = ui * NBT + c
                        nc.tensor.matmul(
                            dst, dg_cur[:, k * 128:(k + 1) * 128],
                            g_tiles[u][:, bass.ts(c, HS)],
                            start=(ui == 0), stop=False)
            else:
                agg_ps = None

            # ---------- agg(w): Pool chain (early) ----------
            pool_part = None
            if pool_us:
                pool_part = hp.tile([128, FLAT], F16, tag="poolpart",
                                    name=f"pp{v}")
                for c in range(NBT):
                    pslab = pool_part[:, bass.ts(c, HS)]
                    for i, u in enumerate(pool_us):
                        gsl = g_tiles[u][:, bass.ts(c, HS)]
                        sc = adj_t[c][:, w * MAXN + u: w * MAXN + u + 1]
                        if i == 0:
                            nc.gpsimd.tensor_scalar(pslab, gsl, sc, None,
                                                    AL.mult)
                        else:
                            nc.gpsimd.scalar_tensor_tensor(
                                pslab, gsl, sc, pslab, AL.mult, AL.add)

            # ---------- agg(w): DVE chain part 1 (early) ----------
            dve_part = None
            if psum_path and (dve1_us or dve2_us):
                dve_part = hp.tile([128, FLAT], F16, tag="dvepart",
                                   name=f"dp{v}")
                for c in range(NBT):
                    dslab = dve_part[:, bass.ts(c, HS)]
                    for i, u in enumerate(dve1_us):
                        gsl = g_tiles[u][:, bass.ts(c, HS)]
                        sc = adj_t[c][:, w * MAXN + u: w * MAXN + u + 1]
                        if i == 0:
                            nc.vector.tensor_scalar(dslab, gsl, sc, None,
                                                    AL.mult)
                        else:
                            nc.vector.scalar_tensor_tensor(
                                dslab, gsl, sc, dslab, AL.mult, AL.add)

            # ---------- GRU matmuls (r, hn interleaved; then z) -------
            rz_blocks = [(xh2[0:XROWS, :], "w_rz0", 0, XROWS)]
            hn_blocks = [(xh2[HIN2:XROWS, :], "w_hnx", HIN2, XROWS)]
            if v > 0:
                rz_blocks += [(hinT0[:], "w_rz1", 0, 128),
                              (hinT1[:], "w_rz2", 0, 128)]
                hn_blocks += [(hinT0[:], "w_hn0", 0, 128),
                              (hinT1[:], "w_hn1", 0, 128)]

            ps_r = [None, None]
            ps_hn = [None, None]
            ps_z = [None, None]

            def gate_mms(ps_pair, j, blocks, col0, col1, nm, pool=None):
                t = (pool or pp).tile([128, 1024], F32,
                                      tag="ps" if pool is None else "agps",
                                      name=f"{nm}{j}_{v}")
                ps_pair[j] = t
                nk = len(blocks)
                for cc in range(2):
                    c = j * 2 + cc
                    dst = t[:, cc * 512:cc * 512 + HS]
                    for k, (st, wn, r0, r1) in enumerate(blocks):
                        nc.tensor.matmul(dst, st[:, bass.ts(c, 128)],
                                         wt[wn][r0:r1, col0:col1],
                                         start=(k == 0), stop=(k == nk - 1))

            # r c01, hn c01, r c23, hn c23, then z (consumed late)
            gate_mms(ps_r, 0, rz_blocks, 0, HS, "psr")
            gate_mms(ps_hn, 0, hn_blocks, 0, HS, "pshn")
            gate_mms(ps_r, 1, rz_blocks, 0, HS, "psr")
            gate_mms(ps_hn, 1, hn_blocks, 0, HS, "pshn")
            gate_mms(ps_z, 0, rz_blocks, HS, 2 * HS, "psz")
            gate_mms(ps_z, 1, rz_blocks, HS, 2 * HS, "psz")

            # ---------- GRU elementwise (c-half pipelined) ----------
            # Act priority: r0, r1, tanh0, z0, tanh1, z1
            r_sb = sp.tile([128, FLAT], F16, tag="rsb")
            z_sb = sp.tile([128, FLAT], F16, tag="zsb")
            q_sb = sp.tile([128, FLAT], F16, tag="qsb")
            t_sb = sp.tile([128, FLAT], F16, tag="tsb")
            n_sb = sp.tile([128, FLAT], F16, tag="nsb")
            d_sb = sp.tile([128, FLAT], F16, tag="dsb")
            e_sb = sp.tile([128, FLAT], F16, tag="esb")
            h_sb = sp.tile([128, FLAT], F16, tag="hsb")
            for j in range(2):
                nc.scalar.activation(sb2(r_sb, j), pv2(ps_r[j]), ACTF.Sigmoid)
            for j in range(2):
                nc.vector.tensor_tensor(sb2(q_sb, j), sb2(r_sb, j),
                                        pv2(ps_hn[j]), AL.mult)
                nc.vector.tensor_tensor(sb2(t_sb, j), sb2(q_sb, j),
                                        sb2(in_t, j), AL.add)

            def gru_tail_half(j):
                nc.scalar.activation(sb2(n_sb, j), sb2(t_sb, j), ACTF.Tanh)
                nc.scalar.activation(sb2(z_sb, j), pv2(ps_z[j]), ACTF.Sigmoid)
                nc.vector.tensor_tensor(sb2(d_sb, j), sb2(hinL2, j),
                                        sb2(n_sb, j), AL.subtract)
                nc.vector.tensor_tensor(sb2(e_sb, j), sb2(z_sb, j),
                                        sb2(d_sb, j), AL.mult)
                nc.vector.tensor_tensor(sb2(h_sb, j), sb2(n_sb, j),
                                        sb2(e_sb, j), AL.add)

            # ---------- transpose h -> L1 (j-half pipelined) ----------
            # separate PSUM tiles per half so each half's copies free its
            # buffer independently (pool-ring pressure)
            hT0 = hp.tile([128, BL], F16, tag="ht0")
            hT1 = hp.tile([128, BL], F16, tag="ht1")
            hT2x = (hp.tile([H2ROWS, BL], F16, tag="ht2", name=f"ht2_{v}")
                    if not last else None)
            head2_t = (hp.tile([HDROWS, BL], F16, tag="head2", name="head2")
                       if last else None)
            if not last:
                nc.sync.dma_start(hT2x[45:H2ROWS, :], post.ap()[v])

            def h_transpose_half(j):
                b0, b1 = j * 256, j * 256 + 256
                tr = pp.tile([128, 1024], F16, tag="ps", name=f"trh{j}_{v}")
                for ki, (k0, k1) in enumerate(HT):
                    kw = k1 - k0
                    for cc in range(2):
                        c = 2 * j + cc
                        nc.tensor.matmul(
                            tr[0:kw, ki * 256 + cc * 128: ki * 256 + cc * 128 + 128],
                            h_sb[:, c * HS + k0: c * HS + k1], id_t[:],
                            is_transpose=True, skip_group_check=True)
                # parallel copies: hT0 on Act, hT1 + h2-rows on DVE
                nc.scalar.copy(hT0[:, b0:b1], tr[0:128, 0:256])
                nc.vector.tensor_copy(hT1[:, b0:b1], tr[0:128, 256:512])
                dst45 = hT2x if not last else head2_t
                nc.vector.tensor_copy(dst45[0:45, b0:b1], tr[0:45, 512:768])

            if last:
                nc.vector.memset(head2_t[32:64, :], 0.0)
                for j in range(2):
                    gru_tail_half(j)
                    h_transpose_half(j)
                nc.vector.tensor_copy(head2_t[64:64 + FEAT, :], hdT_t[:])
                nc.sync.dma_start(head2_t[HDROWS - 1:HDROWS, :], ones1.ap()[:])
                head2 = head2_t
                hT0_last, hT1_last = hT0, hT1
                break

            # ---------- gate/mapper -> g_v; agg finalize (per j-half) ------
            ps_gg = [None, None]
            ps_gm = [None, None]
            g_blocks = [(hT0, "w_g0", 0, 128), (hT1, "w_g1", 0, 128),
                        (hT2x, "w_g2", 0, H2ROWS)]
            m_blocks = [(hT0, "w_m0", 0, 128), (hT1, "w_m1", 0, 128),
                        (hT2x, "w_m2", 0, H2ROWS)]
            sg_sb = sp.tile([128, FLAT], F16, tag="sgsb")
            gv = g_tiles[v]
            hinL2_next = hp.tile([128, FLAT], F16, tag="hinl2")
            hinT0 = hp.tile([128, BL], F16, tag="hinT0")
            hinT1 = hp.tile([128, BL], F16, tag="hinT1")

            # fold partials + backbone into the PSUM accumulator, per c
            started = bool(pe_us)
            folds = []
            if dve_part is not None:
                folds.append(dve_part)
            if pool_part is not None:
                folds.append(pool_part)

            def part_folds(j):
                """dve/pool partial id-folds for half j (ready before gv)."""
                for fi, ft in enumerate(folds):
                    for c in (2 * j, 2 * j + 1):
                        dst = agg_ps[j][:, (c % 2) * 512:(c % 2) * 512 + HS]
                        nc.tensor.matmul(
                            dst, id_t[:], ft[:, bass.ts(c, HS)],
                            start=(not started and fi == 0), stop=False)

            def agg_finalize_half(j):
                """Backbone + PSUM fold for c in {2j, 2j+1}."""
                if psum_path:
                    for c in (2 * j, 2 * j + 1):
                        dst = agg_ps[j][:, (c % 2) * 512:(c % 2) * 512 + HS]
                        nc.tensor.matmul(
                            dst, id_t[:], gv[:, bass.ts(c, HS)],
                            start=(not started and not folds),
                            stop=True)
                    fe = fold_engine(FOLD_ENG) if j == 1 else nc.vector
                    if fe is None:
                        nc.scalar.activation(sb2(hinL2_next, j),
                                             pv2(agg_ps[j]), ACTF.Copy)
                    else:
                        fe.tensor_copy(sb2(hinL2_next, j), pv2(agg_ps[j]))
                else:
                    for c in (2 * j, 2 * j + 1):
                        hslab = hinL2_next[:, bass.ts(c, HS)]
                        egsl = gv[:, bass.ts(c, HS)]
                        if not dve1_us:
                            nc.vector.tensor_copy(hslab, egsl)
                        else:
                            for i, u in enumerate(dve1_us):
                                gsl = g_tiles[u][:, bass.ts(c, HS)]
                                sc = adj_t[c][:, w * MAXN + u:
                                              w * MAXN + u + 1]
                                if i == 0:
                                    nc.vector.tensor_scalar(
                                        hslab, gsl, sc, None, AL.mult)
                                else:
                                    nc.vector.scalar_tensor_tensor(
                                        hslab, gsl, sc, hslab, AL.mult, AL.add)
                            nc.vector.tensor_tensor(hslab, egsl, hslab, AL.add)

            def hin_transpose_half(j):
                b0, b1 = j * 256, j * 256 + 256
                tr = pp.tile([128, 1024], F16, tag="ps", name=f"trn{j}_{v}")
                for ki, (k0, k1) in enumerate(HT):
                    kw = k1 - k0
                    for cc in range(2):
                        c = 2 * j + cc
                        nc.tensor.matmul(
                            tr[0:kw, ki * 256 + cc * 128: ki * 256 + cc * 128 + 128],
                            hinL2_next[:, c * HS + k0: c * HS + k1], id_t[:],
                            is_transpose=True, skip_group_check=True)
                # parallel copies: hinT0 on DVE, hinT1 on Act, h2-rows on DVE
                nc.vector.tensor_copy(hinT0[:, b0:b1], tr[0:128, 0:256])
                nc.scalar.copy(hinT1[:, b0:b1], tr[0:128, 256:512])
                nc.vector.tensor_copy(xh2_next[HIN2:HIN2 + 45, b0:b1],
                                      tr[0:45, 512:768])

            # half 0 through the whole tail, then half 1
            gru_tail_half(0)
            h_transpose_half(0)
            gru_tail_half(1)
            gate_mms(ps_gg, 0, g_blocks, 0, HS, "psgg")
            gate_mms(ps_gm, 0, m_blocks, 0, HS, "psgm")
            nc.scalar.activation(sb2(sg_sb, 0), pv2(ps_gg[0]), ACTF.Sigmoid)
            nc.vector.tensor_tensor(sb2(gv, 0), sb2(sg_sb, 0),
                                    pv2(ps_gm[0]), AL.mult)
            h_transpose_half(1)
            gate_mms(ps_gg, 1, g_blocks, 0, HS, "psgg")
            nc.scalar.activation(sb2(sg_sb, 1), pv2(ps_gg[1]), ACTF.Sigmoid)
            # DVE agg chain part 2 fills the gate-matmul window
            if dve2_us:
                for c in range(NBT):
                    dslab = dve_part[:, bass.ts(c, HS)]
                    for i, u in enumerate(dve2_us):
                        gsl = g_tiles[u][:, bass.ts(c, HS)]
                        sc = adj_t[c][:, w * MAXN + u: w * MAXN + u + 1]
                        if not dve1_us and i == 0:
                            nc.vector.tensor_scalar(dslab, gsl, sc, None,
                                                    AL.mult)
                        else:
                            nc.vector.scalar_tensor_tensor(
                                dslab, gsl, sc, dslab, AL.mult, AL.add)
            if psum_path and folds:
                part_folds(0)
                part_folds(1)
            agg_finalize_half(0)
            hin_transpose_half(0)
            # mapper j1 PSUM lives in the agps ring so the pp ring lets the
            # next step's r-matmuls start off the j0 tail
            gate_mms(ps_gm, 1, m_blocks, 0, HS, "psgm", pool=ap_)
            nc.vector.tensor_tensor(sb2(gv, 1), sb2(sg_sb, 1),
                                    pv2(ps_gm[1]), AL.mult)
            agg_finalize_half(1)
            hin_transpose_half(1)

            xh2 = xh2_next
            in_t = in_t_next
            hinL2 = hinL2_next
            dg_cur = dg_next

        # ---------- head ----------
        ps_o = [pp.tile([128, 1024], F32, tag="ps", name=f"pso{j}")
                for j in range(2)]
        hd_blocks = [(hT0_last, "w_hd0"), (hT1_last, "w_hd1"),
                     (head2, "w_hd2")]
        for c in range(NBT):
            dst = ps_o[c // 2][:, (c % 2) * 512:(c % 2) * 512 + 112]
            for k, (st, wn) in enumerate(hd_blocks):
                nc.tensor.matmul(dst, st[:, bass.ts(c, 128)], wt[wn][:],
                                 start=(k == 0), stop=(k == 2))
        out_sb = sp.tile([128, NBT * 112], F32, tag="outsb")
        for j in range(2):
            nc.scalar.activation(
                out_sb[:, j * 224:(j + 1) * 224].rearrange(
                    "p (c w) -> p c w", c=2),
                ps_o[j][:].rearrange("p (c w) -> p c w", c=2)[:, :, 0:112],
                ACTF.Copy)
        nc.sync.dma_start(
            out_d.ap().rearrange("c p w -> p c w"),
            out_sb[:].rearrange("p (c w) -> p c w", c=NBT))
    if CAP_WAITS:
        _cap_sync_waits(nc)
    return nc


def _cap_sync_waits(nc, maxw=2):
    """Walrus codegen in this build supports at most `maxw` sem waits per
    instruction (1 for Drain/NoOp ctrl structs).  Move overflow waits onto
    same-engine NoOp instructions inserted immediately before."""
    fn = nc.m.functions[0]
    nid = [0]
    for bb in fn.blocks:
        insts = list(bb.instructions)
        out = []
        for inst in insts:
            si = inst.sync_info
            waits = list(si.on_wait) if si and si.on_wait else []
            limit = 1
            if len(waits) > limit:
                keep = waits[len(waits) - limit:]
                extra = waits[:len(waits) - limit]
                for w in extra:
                    nop = mybir.InstNoOp(name=f"WCAP-{nid[0]}")
                    nid[0] += 1
                    nop.engine = inst.engine
                    nop.sync_info = mybir.SyncInfo(on_wait=[w], on_update=[])
                    out.append(nop)
                si.on_wait = keep
            out.append(inst)
        bb.instructions = out


def _make_in_maps(inp):
    W = _prep_weights(inp)
    f16 = np.float16

    types_, pos_ = inp["types"], inp["pos"]
    X = np.zeros((B, MAXN, 65), f16)
    X[np.arange(B)[:, None], np.arange(MAXN)[None, :], types_] = 1
    X[np.arange(B)[:, None], np.arange(MAXN)[None, :], NVT + pos_] = 1
    X[:, :, XD] = 1.0    # ones row for rz bias

    pos_oh = np.zeros((B, MAXN, 10), f16)
    pos_oh[np.arange(B)[:, None], np.arange(MAXN)[None, :], pos_] = 1
    pos_oh[:, :, 9] = 1.0  # ones row (gate bias)

    # host-gathered i_n = W_ihn[:, type] + W_ihn[:, 26+pos] + b_ihn, as a
    # [234, 301] combined table indexed by type*9+pos
    Wih_n = inp["W_ih"][2 * HS:3 * HS]                        # [301, 35]
    b_ihn = inp["b_ih"][2 * HS:3 * HS]
    combo = (Wih_n[:, :NVT][:, :, None] + Wih_n[:, NVT:][:, None, :]
             + b_ihn[:, None, None])                          # [301, 26, 9]
    combo = combo.reshape(HS, NVT * P9)                       # [301, 234]
    idx = (np.asarray(types_) * P9 + np.asarray(pos_))        # [B, MAXN]
    in_full = combo.T[idx]                                    # [B, MAXN, 301]

    adjf = inp["adj"].astype(np.float32)
    hdf_ = inp["hdf"].astype(np.float32)
    terms, step_off, step_cnt = _pe_terms()

    in_maps = []
    ar = np.arange(128)
    for core in range(NCORES):
        sl = slice(core * BL, (core + 1) * BL)
        m = {}
        m["xt"] = np.ascontiguousarray(X[sl].transpose(1, 2, 0))
        m["post"] = np.ascontiguousarray(pos_oh[sl].transpose(1, 2, 0))
        # int_[v, p, c*301:] = i_n for batch row (c*128 + p)
        inc = in_full[sl].reshape(NBT, 128, MAXN, HS)
        m["int_"] = np.ascontiguousarray(
            inc.transpose(2, 1, 0, 3).reshape(MAXN, 128, FLAT).astype(f16))
        m["ones1"] = np.ones((1, BL), f16)
        m["adjt"] = np.ascontiguousarray(adjf[sl].reshape(NBT, 128, MAXN * MAXN))
        adj_core = adjf[sl].reshape(NBT, 128, MAXN, MAXN)
        npe4 = max(len(terms), 1)
        dg = np.zeros((128, npe4, 128), f16)
        for k, (wv, u, c) in enumerate(terms):
            dg[ar, k, ar] = adj_core[c, :, wv, u]
        m["diagall"] = np.ascontiguousarray(dg.reshape(128, npe4 * 128))
        hd = np.zeros((28, BL), f16)
        hd[0:27] = hdf_[sl].T.astype(f16)
        hd[27] = 1.0
        m["hdft"] = hd
        m["ident"] = np.eye(128, dtype=f16)
        for k, v in W.items():
            m[k] = np.ascontiguousarray(v)
        in_maps.append(m)
    return in_maps


_CACHE = {}


def _get_nc():
    _patch_tile_drain()
    if "nc" not in _CACHE:
        nc = bass.Bass("TRN2", target_bir_lowering=False, debug=False)
        _build(nc)
        _CACHE["nc"] = nc
    return _CACHE["nc"]


def kernel(types, pos, adj, hdf, Wg, bg, Wm, W_ih, W_hh, b_ih, b_hh,
           Wd1, bd1, Wd2, bd2, Wmu, bmu, Wlv, blv, _return_nc=False):
    inp = dict(types=types, pos=pos, adj=adj, hdf=hdf, Wg=Wg, bg=bg, Wm=Wm,
               W_ih=W_ih, W_hh=W_hh, b_ih=b_ih, b_hh=b_hh, Wd1=Wd1, bd1=bd1,
               Wd2=Wd2, bd2=bd2, Wmu=Wmu, bmu=bmu, Wlv=Wlv, blv=blv)
    inp = {k: np.asarray(v) for k, v in inp.items()}
    in_maps = _make_in_maps(inp)
    nc = _get_nc()

    res = run_bass_kernel_spmd(nc, in_maps, list(range(NCORES)))
    mu = np.zeros((B, NZ), np.float32)
    lv = np.zeros((B, NZ), np.float32)
    for core in range(NCORES):
        o = res.results[core]["out"].reshape(BL, 112)
        sl = slice(core * BL, (core + 1) * BL)
        mu[sl] = o[:, 0:NZ]
        lv[sl] = o[:, NZ:112]
    if _return_nc:
        return (mu, lv), res
    return mu, lv
